# revision 16
# baseline (speedup 1.0000x reference)
"""Trainium2 Bass kernel for AttentionWithRotaryPositionalEmbedding (v2).

Shapes (hardcoded): x [4, 2048, 512], 8 heads, head dim 64.
Sharding: 8 cores = (batch b = core//2) x (query half = core%2); k/v computed
locally from the full x[b] so no collectives.

v2 design (measured-primitive driven):
- Scores K=64 matmuls for the two heads of a ct-block are ROW-TILED
  (lhsT/rhs partition bases 0 and 64 -> tile_position (0,0)/(64,0)) and run
  concurrently on the PE: measured 108 ns/MM vs 427 serial.
- The EXP stream on ACT is the true bottleneck (128 x ~1.15us, dtype/scale
  independent). The schedule keeps ACT 100% busy; PE work (cut from 752 to
  ~590 matmuls) fits in the gaps.
- RoPE rotation via DVE stream_shuffle (partition pair-swap, sin table holds
  the sign) instead of PE matmuls; softmax-denominator replication via K=1
  matmul pairs; output-projection bias folded into the DVE ysum add against
  a host-replicated bias tile.
- attn@v per head accumulates [65, NQ] in PSUM (row 64 = denominators);
  heads run E(even) then O(odd) per pair with the O ex-stream buffered
  (delayed-O) so a single 2-bank psav suffices; the LAST pair runs both
  heads interleaved with the O head accumulating in the 2 pgen banks
  (free by then) so there is no tail burst.
- Inputs host-packed into 6 dram tensors so the critical path is 2 big DMAs;
  output stored f16 in 2 DMAs and unscrambled on host.
"""

import sys

import numpy as np

if "/opt/trn_rl_repo" not in sys.path:
    sys.path.insert(0, "/opt/trn_rl_repo")

B, N, C = 4, 2048, 512
H, DH = 8, 64
NQ = 1024
P = 128
NCH = 16
VW = DH + 1
MAX_FPS = np.float32(30.0)

_CACHE = {}


def _host_prep(x, mask, times, Wqkv, Wproj, bproj):
    x = np.asarray(x, np.float32)
    mask = np.asarray(mask, np.float32)
    times = np.asarray(times, np.float32)
    Wqkv = np.asarray(Wqkv, np.float32)
    Wproj = np.asarray(Wproj, np.float32)
    bproj = np.asarray(bproj, np.float32)

    wtT = np.ascontiguousarray(Wqkv.T).astype(np.float16)    # [512, 1536]
    wptT = np.ascontiguousarray(Wproj.T).astype(np.float16)  # [512, 512]

    # wt packed [128, 6144]: g0 = per-ci [q-ct0 | k-ct0] (cols 0:1024),
    # g1 = per-ci v (1024:3072), g2 = per-ci [q-ct1:3 | k-ct1:3] (3072:6144)
    w4 = wtT.reshape(4, P, 1536)
    wt_h = np.empty((P, 6144), np.float16)
    for ci in range(4):
        wt_h[:, ci * 256:ci * 256 + 128] = w4[ci][:, 0:128]
        wt_h[:, ci * 256 + 128:ci * 256 + 256] = w4[ci][:, 512:640]
        wt_h[:, 1024 + ci * 512:1024 + (ci + 1) * 512] = w4[ci][:, 1024:1536]
        wt_h[:, 3072 + ci * 768:3072 + ci * 768 + 384] = w4[ci][:, 128:512]
        wt_h[:, 3072 + ci * 768 + 384:3072 + (ci + 1) * 768] = w4[ci][:, 640:1024]

    wpt_h = np.empty((P, 2048), np.float16)  # [hp block][512 out-cols]
    for hp in range(4):
        wpt_h[:, hp * 512:(hp + 1) * 512] = wptT[hp * 128:(hp + 1) * 128, :]

    inv_freq = (np.float32(1.0) /
                (np.float32(10000.0) **
                 (np.arange(0, DH, 2, dtype=np.float32) / np.float32(DH))))
    pos = np.round(times * MAX_FPS)               # [B, N], round-half-even
    ridx = (np.arange(P) % DH) // 2               # row -> pair index
    sgn = np.where(np.arange(P) % 2 == 0, 1.0, -1.0).astype(np.float32)[:, None]

    in_maps = []
    for core in range(8):
        b, qhalf = core // 2, core % 2
        perm = np.arange(N) if qhalf == 0 else np.r_[NQ:N, 0:NQ]
        xb = x[b].T[:, perm]                      # [512, 2048]
        # xt packed [128, (nb, ci, 512)]
        xt_h = np.ascontiguousarray(
            xb.reshape(4, P, 4, 512).transpose(1, 2, 0, 3).reshape(P, 8192)
        ).astype(np.float16)
        freqs = pos[b][perm][None, :] * inv_freq[:, None]   # [32, 2048]
        cos32 = np.cos(freqs)
        sin32 = np.sin(freqs)
        ce_h = np.empty((P, 4096), np.float32)
        ce_h[:, 0:2048] = cos32[ridx]
        ce_h[:, 2048:4096] = sin32[ridx] * sgn    # sign folded for pair-swap
        ce_h = np.ascontiguousarray(ce_h.astype(np.float16))
        em = np.exp(mask[b][perm]).astype(np.float32)
        emask_h = np.ascontiguousarray(em.reshape(NCH, P).T)  # [128, 16]
        bias_h = np.zeros((P, 576), np.float16)
        bias_h[:, 0:512] = bproj[None, :].astype(np.float16)
        bias_h[:, 512:576] = np.float16(1.0)      # ones row for den replicate
        in_maps.append({"xt": xt_h, "wt": wt_h, "wpt": wpt_h, "ce": ce_h,
                        "emask": emask_h, "bias": bias_h})
    return in_maps


def _build_module():
    import concourse.tile as tile
    import concourse.mybir as mybir
    from concourse import bacc

    f32 = mybir.dt.float32
    f16 = mybir.dt.float16
    EXPF = mybir.ActivationFunctionType.Exp
    nc = bacc.Bacc(None, target_bir_lowering=False, debug=False)

    xt_d = nc.dram_tensor("xt", [P, 8192], f16, kind="ExternalInput")
    wt_d = nc.dram_tensor("wt", [P, 6144], f16, kind="ExternalInput")
    wpt_d = nc.dram_tensor("wpt", [P, 2048], f16, kind="ExternalInput")
    ce_d = nc.dram_tensor("ce", [P, 4096], f16, kind="ExternalInput")
    emask_d = nc.dram_tensor("emask", [P, NCH], f32, kind="ExternalInput")
    bias_d = nc.dram_tensor("bias", [P, 576], f16, kind="ExternalInput")
    y_d = nc.dram_tensor("y", [P, 4096], f16, kind="ExternalOutput")

    MM = nc.tensor.matmul
    COPYF = mybir.ActivationFunctionType.Copy
    SWAPM = [i ^ 1 for i in range(32)]

    # virtual-time cost estimates (us), tuned to measured primitives
    EXPT = 1.15      # EXP [128,1024] on ACT
    S_CH = 0.46      # one chunk of row-tiled scores (4 MMs as 2 pairs)
    AV = 0.52        # attn@v one head-chunk (2 MMs + ldw)
    CHAIN4 = 0.97    # 4-MM K=128 accumulation chain
    DEN2 = 0.52
    OP2 = 0.52
    PCAST, MULF, SHUF, GPADD, ADDF = 0.45, 0.45, 0.70, 1.25, 0.46
    VEVACA = 0.75    # ACT copy [128,512] with per-partition scale
    SCC, RECIP, NMUL = 0.72, 1.25, 1.25

    with tile.TileContext(nc) as tc:
        with (
            tc.tile_pool(name="consts", bufs=1) as consts,
            tc.tile_pool(name="big", bufs=1) as big,
            tc.tile_pool(name="exEp", bufs=10) as exEp,
            tc.tile_pool(name="exOp", bufs=20) as exOp,
            tc.tile_pool(name="tmps", bufs=2) as tmps,
            tc.tile_pool(name="prawp", bufs=2) as prawp,
            tc.tile_pool(name="shfp", bufs=2) as shfp,
            tc.tile_pool(name="scpool", bufs=3) as scpool,
            tc.tile_pool(name="rrp", bufs=2) as rrp,
            tc.tile_pool(name="pss", bufs=2, space="PSUM") as pss,
            tc.tile_pool(name="psavp", bufs=1, space="PSUM") as psavp,
            tc.tile_pool(name="pgen", bufs=2, space="PSUM") as pgen,
        ):
            wt_s = consts.tile([P, 6144], f16, name="wt_s")
            xt_s = consts.tile([P, 4, 4, 512], f16, name="xt_s")
            ce_s = consts.tile([P, 4096], f16, name="ce_s")
            wpt_s = consts.tile([P, 2048], f16, name="wpt_s")
            bias_s = consts.tile([P, 576], f16, name="bias_s")
            emask_s = consts.tile([P, NCH], f32, name="emask_s")

            qT = [big.tile([P, NQ], f16, name=f"qT{i}") for i in range(4)]
            kT = [big.tile([P, N], f16, name=f"kT{i}") for i in range(4)]
            v65 = big.tile([P, NCH, H, VW], f16, name="v65")
            scp = [big.tile([P, NQ], f16, name=f"scp{i}") for i in range(4)]
            ysum = big.tile([P, 8, 512], f16, name="ysum")
            y_all = big.tile([P, 8, 512], f16, name="y_all")

            # ---------------- DMAs (priority order) ----------------
            DMA = nc.sync.dma_start
            DMA(wt_s[:, 0:1024], wt_d.ap()[:, 0:1024])
            DMA(xt_s[:, 0, :, :], xt_d.ap()[:, 0:2048])
            DMA(ce_s[:, 0:1024], ce_d.ap()[:, 0:1024])
            DMA(ce_s[:, 2048:3072], ce_d.ap()[:, 2048:3072])
            DMA(xt_s[:, 1, :, :], xt_d.ap()[:, 2048:4096])
            DMA(emask_s[:], emask_d.ap())
            DMA(bias_s[:], bias_d.ap())
            DMA(wt_s[:, 1024:3072], wt_d.ap()[:, 1024:3072])
            DMA(xt_s[:, 2, :, :], xt_d.ap()[:, 4096:6144])
            DMA(xt_s[:, 3, :, :], xt_d.ap()[:, 6144:8192])
            DMA(ce_s[:, 1024:2048], ce_d.ap()[:, 1024:2048])
            DMA(ce_s[:, 3072:4096], ce_d.ap()[:, 3072:4096])
            DMA(wt_s[:, 3072:6144], wt_d.ap()[:, 3072:6144])
            DMA(wpt_s[:], wpt_d.ap())

            def wt_q(ci, ct):
                o = ci * 256 if ct == 0 else 3072 + ci * 768 + (ct - 1) * 128
                return wt_s[:, o:o + 128]

            def wt_k(ci, ct):
                o = (ci * 256 + 128 if ct == 0
                     else 3072 + ci * 768 + 384 + (ct - 1) * 128)
                return wt_s[:, o:o + 128]

            def wt_v(ci):
                o = 1024 + ci * 512
                return wt_s[:, o:o + 512]

            # ---------------- state ----------------
            st = {"pe": 0.0, "act": 0.0, "dve": 0.0, "gp": 0.0}
            kq_ready = {}
            v_ready = {}
            exp_done = {}
            ex_tiles = {}
            sccopy_done = {}
            norm_done = {}
            den_jobs = []

            # ---------------- emitters ----------------
            def chain_qk(sp, ct, nb, dve_add=False):
                pg = pgen.tile([P, 512], f32, name="pg")
                wf = wt_q if sp == "q" else wt_k
                for ci in range(4):
                    MM(pg[:], wf(ci, ct), xt_s[:, nb, ci, :],
                       start=(ci == 0), stop=(ci == 3))
                st["pe"] += CHAIN4
                tok = nb * 512
                praw = prawp.tile([P, 512], f16, name="praw")
                nc.vector.tensor_copy(praw[:], pg[:])
                t = tmps.tile([P, 2, 512], f16, name="t")
                nc.vector.tensor_mul(t[:, 0, :], praw[:], ce_s[:, tok:tok + 512])
                nc.vector.tensor_mul(t[:, 1, :], praw[:],
                                     ce_s[:, 2048 + tok:2560 + tok])
                ts = shfp.tile([P, 512], f16, name="ts")
                nc.vector.stream_shuffle(ts[:], t[:, 1, :], SWAPM)
                st["dve"] = max(st["dve"], st["pe"]) + PCAST + 2 * MULF + SHUF
                dest = qT[ct] if sp == "q" else kT[ct]
                if dve_add:
                    nc.vector.tensor_add(dest[:, tok:tok + 512], t[:, 0, :], ts[:])
                    st["dve"] += ADDF
                    kq_ready[(sp, ct, nb)] = st["dve"]
                else:
                    nc.gpsimd.tensor_add(dest[:, tok:tok + 512], t[:, 0, :], ts[:])
                    st["gp"] = max(st["gp"], st["dve"]) + GPADD
                    kq_ready[(sp, ct, nb)] = st["gp"]

            def chain_v(c):
                pg = pgen.tile([P, 512], f32, name="pg")
                nb, off = c // 4, (c % 4) * 128
                for ci in range(4):
                    MM(pg[:], xt_s[:, nb, ci, off:off + 128], wt_v(ci),
                       start=(ci == 0), stop=(ci == 3))
                st["pe"] += CHAIN4
                vv = v65[:, c, :, :]
                nc.scalar.activation(
                    vv[:, :, 0:DH],
                    pg[:].rearrange("p (h w) -> p h w", w=DH),
                    COPYF, scale=emask_s[:, c:c + 1])
                st["act"] = max(st["act"], st["pe"]) + VEVACA
                nc.vector.tensor_copy(
                    vv[:, :, DH:DH + 1],
                    emask_s[:, c:c + 1, None].to_broadcast((P, H, 1)))
                st["dve"] = max(st["dve"], st["pe"]) + 0.2
                v_ready[c] = max(st["act"], st["dve"])

            def emit_s_exp(p, c):
                kc = kT[p][:, c * 128:(c + 1) * 128]
                need = max(kq_ready.get(("q", p, 0), 0.0),
                           kq_ready.get(("q", p, 1), 0.0),
                           kq_ready.get(("k", p, c // 4), 0.0))
                st["pe"] = max(st["pe"], need)
                psE = pss.tile([P, NQ], f32, name="ps")
                psO = pss.tile([P, NQ], f32, name="ps")
                for qb in (0, 1):
                    MM(psE[:, qb * 512:(qb + 1) * 512], kc[0:64, :],
                       qT[p][0:64, qb * 512:(qb + 1) * 512],
                       start=True, stop=True)
                    MM(psO[:, qb * 512:(qb + 1) * 512], kc[64:128, :],
                       qT[p][64:128, qb * 512:(qb + 1) * 512],
                       start=True, stop=True)
                st["pe"] += S_CH
                hE, hO = 2 * p, 2 * p + 1
                exE = exEp.tile([P, NQ], f16, name="exE")
                nc.scalar.activation(exE[:], psE[:], EXPF, scale=0.125)
                st["act"] = max(st["act"], st["pe"]) + EXPT
                exp_done[(hE, c)] = st["act"]
                ex_tiles[(hE, c)] = exE
                exO = exOp.tile([P, NQ], f16, name="exO")
                nc.scalar.activation(exO[:], psO[:], EXPF, scale=0.125)
                st["act"] += EXPT
                exp_done[(hO, c)] = st["act"]
                ex_tiles[(hO, c)] = exO

            psav_t = [None]
            pg_o3 = [None, None]

            def emit_av(h, c):
                ex = ex_tiles.pop((h, c))
                if h == 7:
                    if c == 0:
                        pg_o3[0] = pgen.tile([P, 512], f32, name="pg")
                        pg_o3[1] = pgen.tile([P, 512], f32, name="pg")
                    for qb in (0, 1):
                        MM(pg_o3[qb][0:VW, :], v65[:, c, h, :],
                           ex[:, qb * 512:(qb + 1) * 512],
                           start=(c == 0), stop=(c == 15))
                else:
                    if c == 0:
                        psav_t[0] = psavp.tile([VW, NQ], f32, name="ps_o")
                    po = psav_t[0]
                    for qb in (0, 1):
                        MM(po[:, qb * 512:(qb + 1) * 512], v65[:, c, h, :],
                           ex[:, qb * 512:(qb + 1) * 512],
                           start=(c == 0), stop=(c == 15))
                st["pe"] += AV

            def start_epilogue(h):
                sc = scpool.tile([VW, NQ], f16, name="sc")
                if h == 7:
                    for qb in (0, 1):
                        nc.vector.tensor_copy(
                            sc[:, qb * 512:(qb + 1) * 512], pg_o3[qb][0:VW, :])
                    st["dve"] = max(st["dve"], st["pe"]) + SCC
                else:
                    po = psav_t[0]
                    nc.vector.tensor_copy(sc[:], po[:])
                    st["dve"] = max(st["dve"], st["pe"]) + SCC
                sccopy_done[h] = st["dve"]
                den_jobs.append({"h": h, "sc": sc, "ready": st["dve"]})

            def emit_den(job):
                h, sc = job["h"], job["sc"]
                p, off = h // 2, (h % 2) * 64
                st["pe"] = max(st["pe"], job["ready"])
                ones = bias_s[64:65, 512:576]
                rr = rrp.tile([DH, NQ], f32, name="rr")
                if h < 4:
                    pd0 = pgen.tile([P, 512], f32, name="pg")
                    pd1 = pgen.tile([P, 512], f32, name="pg")
                    MM(pd0[0:64, :], ones, sc[64:65, 0:512], start=True, stop=True)
                    MM(pd1[0:64, :], ones, sc[64:65, 512:1024], start=True, stop=True)
                    st["pe"] += DEN2
                    nc.vector.reciprocal_approx_fast(rr[:, 0:512], pd0[0:64, :])
                    nc.vector.reciprocal_approx_fast(rr[:, 512:1024], pd1[0:64, :])
                else:
                    pd = pss.tile([P, NQ], f32, name="ps")
                    MM(pd[0:64, 0:512], ones, sc[64:65, 0:512], start=True, stop=True)
                    MM(pd[0:64, 512:1024], ones, sc[64:65, 512:1024],
                       start=True, stop=True)
                    st["pe"] += DEN2
                    nc.vector.reciprocal_approx_fast(rr[:], pd[0:64, :])
                nc.vector.tensor_mul(scp[p][off:off + 64, :], sc[0:64, :], rr[:])
                st["dve"] = max(st["dve"], st["pe"]) + RECIP + NMUL
                norm_done[h] = st["dve"]

            def emit_opA(nbk):
                pg = pgen.tile([P, 512], f32, name="pg")
                MM(pg[:], scp[0][:, nbk * 128:(nbk + 1) * 128],
                   wpt_s[:, 0:512], start=True, stop=False)
                MM(pg[:], scp[1][:, nbk * 128:(nbk + 1) * 128],
                   wpt_s[:, 512:1024], start=False, stop=True)
                st["pe"] += OP2
                nc.vector.tensor_add(ysum[:, nbk, :], pg[:], bias_s[:, 0:512])
                st["dve"] = max(st["dve"], st["pe"]) + NMUL

            def emit_tail(nbk, sl):
                MM(sl, scp[2][:, nbk * 128:(nbk + 1) * 128],
                   wpt_s[:, 1024:1536], start=True, stop=False)
                MM(sl, scp[3][:, nbk * 128:(nbk + 1) * 128],
                   wpt_s[:, 1536:2048], start=False, stop=True)
                st["pe"] += OP2
                nc.vector.tensor_add(y_all[:, nbk, :], ysum[:, nbk, :], sl)
                st["dve"] = max(st["dve"], st["pe"]) + NMUL

            # ---------------- schedule ----------------
            chain_qk("q", 0, 0, dve_add=True)
            chain_qk("k", 0, 0, dve_add=True)
            chain_qk("q", 0, 1, dve_add=True)

            filler = [("k", 0, 1), ("v", 0), ("k", 0, 2), ("v", 1),
                      ("k", 0, 3), ("v", 2), ("q", 1, 0), ("v", 3),
                      ("q", 1, 1), ("v", 4), ("k", 1, 0), ("v", 5),
                      ("k", 1, 1), ("v", 6), ("k", 1, 2), ("v", 7),
                      ("k", 1, 3), ("v", 8), ("v", 9), ("q", 2, 0),
                      ("v", 10), ("q", 2, 1), ("v", 11), ("k", 2, 0),
                      ("v", 12), ("k", 2, 1), ("v", 13), ("k", 2, 2),
                      ("v", 14), ("k", 2, 3), ("v", 15), ("q", 3, 0),
                      ("q", 3, 1), ("k", 3, 0), ("k", 3, 1), ("k", 3, 2),
                      ("k", 3, 3)]
            fpos = {it: i for i, it in enumerate(filler)}
            fidx = [0]

            def consume_one():
                it = filler[fidx[0]]
                fidx[0] += 1
                if it[0] == "v":
                    chain_v(it[1])
                else:
                    chain_qk(*it)

            def consume_until(i):
                while fidx[0] <= min(i, len(filler) - 1):
                    consume_one()

            def req_s(p, c):
                mx = -1
                for key in (("q", p, 0), ("q", p, 1), ("k", p, c // 4)):
                    if key in fpos:
                        mx = max(mx, fpos[key])
                return mx

            avq = []
            for p in range(3):
                avq += [(2 * p, c) for c in range(NCH)]
                avq += [(2 * p + 1, c) for c in range(NCH)]
            for c in range(NCH):
                avq += [(6, c), (7, c)]
            avi = [0]
            # psav users in order; each gated on predecessor's sccopy
            psav_prev = {0: None, 1: 0, 2: 1, 3: 2, 4: 3, 5: 4, 6: 5}
            opA_q = []
            opA_state = [0]  # 0=locked, 1=unlocked

            def av_ready(slack):
                if avi[0] >= len(avq):
                    return False
                h, c = avq[avi[0]]
                if exp_done.get((h, c), 1e9) > st["pe"] + slack:
                    return False
                if v_ready.get(c, 1e9) > st["pe"] + slack:
                    return False
                if c == 0:
                    if h == 7:
                        # pgen claim: all pgen users must be emitted first
                        if fidx[0] < len(filler) or opA_q or opA_state[0] == 0:
                            return False
                        if any(j["h"] < 4 for j in den_jobs):
                            return False
                    else:
                        prev = psav_prev[h]
                        if prev is not None and \
                                sccopy_done.get(prev, 1e9) > st["pe"] + slack:
                            return False
                return True

            def try_unlock_opA():
                if opA_state[0] == 0 and norm_done.get(3, 1e9) <= st["dve"]:
                    opA_state[0] = 1
                    opA_q.extend(range(8))

            def emit_one_av():
                h, c = avq[avi[0]]
                avi[0] += 1
                emit_av(h, c)
                if c == 15:
                    start_epilogue(h)

            for s in range(64):
                p, c = divmod(s, NCH)
                # paced filler: walk toward the requirement 4 periods out,
                # at most 2 chains per period (avoids bursts that stall EXP)
                tgt = req_s(*divmod(min(s + 4, 63), NCH))
                n = 0
                while fidx[0] <= tgt and n < 2:
                    consume_one()
                    n += 1
                consume_until(req_s(p, c))
                emit_s_exp(p, c)
                while den_jobs and den_jobs[0]["ready"] <= st["pe"] + 0.6:
                    emit_den(den_jobs.pop(0))
                try_unlock_opA()
                # attnv: keep pace with EXP production (2 head-chunks per
                # period) plus one catch-up when the backlog grows
                backlog = 2 * (s + 1) - avi[0]
                nav = 2 + (1 if backlog > 8 else 0)
                navper = 0
                while navper < nav and av_ready(0.3):
                    emit_one_av()
                    navper += 1
                while st["pe"] < st["act"] - 0.1:
                    if av_ready(0.3):
                        emit_one_av()
                    elif den_jobs and den_jobs[0]["ready"] <= st["pe"]:
                        emit_den(den_jobs.pop(0))
                    elif opA_q:
                        emit_opA(opA_q.pop(0))
                        try_unlock_opA()
                    elif fidx[0] < len(filler):
                        consume_one()
                    else:
                        break

            # ---------------- drain ----------------
            guard = 0
            while avi[0] < len(avq):
                guard += 1
                assert guard < 10000, "drain stall"
                try_unlock_opA()
                if av_ready(0.3):
                    emit_one_av()
                elif den_jobs and den_jobs[0]["ready"] <= st["pe"] + 0.3:
                    emit_den(den_jobs.pop(0))
                elif fidx[0] < len(filler):
                    consume_one()
                elif opA_q and opA_state[0] == 1:
                    emit_opA(opA_q.pop(0))
                else:
                    st["pe"] += 0.25  # idle nudge toward blocking dep
            while fidx[0] < len(filler):
                consume_one()
            while den_jobs:
                emit_den(den_jobs.pop(0))
            try_unlock_opA()
            while opA_q:
                emit_opA(opA_q.pop(0))

            for g in range(4):
                pt = pss.tile([P, NQ], f32, name="ps")
                emit_tail(2 * g, pt[:, 0:512])
                emit_tail(2 * g + 1, pt[:, 512:1024])
                if g == 1:
                    DMA(y_d.ap()[:, 0:2048], y_all[:, 0:4, :])
            DMA(y_d.ap()[:, 2048:4096], y_all[:, 4:8, :])

    nc.compile()
    return nc


def _get_module():
    if "nc" not in _CACHE:
        _CACHE["nc"] = _build_module()
    return _CACHE["nc"]


def kernel(x, mask, times, Wqkv, Wproj, bproj, num_cls_token=0, _trace=False):
    from concourse.bass_utils import run_bass_kernel_spmd

    assert int(num_cls_token) == 0, "kernel specialized for num_cls_token=0"
    in_maps = _host_prep(x, mask, times, Wqkv, Wproj, bproj)
    nc = _get_module()
    res = run_bass_kernel_spmd(nc, in_maps, list(range(8)), trace=_trace)
    _CACHE["last_result"] = res

    out = np.empty((B, N, C), np.float32)
    for core in range(8):
        b, qhalf = core // 2, core % 2
        y = np.asarray(res.results[core]["y"], np.float32)   # [128, 4096]
        blk = y.reshape(P, 8, 512).transpose(1, 0, 2).reshape(NQ, C)
        out[b, qhalf * NQ:(qhalf + 1) * NQ, :] = blk
    return out


# revision 18
# speedup vs baseline: 1.0415x; 1.0415x over previous
"""Trainium2 Bass kernel for AttentionWithRotaryPositionalEmbedding (v2).

Shapes (hardcoded): x [4, 2048, 512], 8 heads, head dim 64.
Sharding: 8 cores = (batch b = core//2) x (query half = core%2); k/v computed
locally from the full x[b] so no collectives.

v2 design (measured-primitive driven):
- Scores K=64 matmuls for the two heads of a ct-block are ROW-TILED
  (lhsT/rhs partition bases 0 and 64 -> tile_position (0,0)/(64,0)) and run
  concurrently on the PE: measured 108 ns/MM vs 427 serial.
- The EXP stream on ACT is the true bottleneck (128 x ~1.15us, dtype/scale
  independent). The schedule keeps ACT 100% busy; PE work (cut from 752 to
  ~590 matmuls) fits in the gaps.
- RoPE rotation via DVE stream_shuffle (partition pair-swap, sin table holds
  the sign) instead of PE matmuls; softmax-denominator replication via K=1
  matmul pairs; output-projection bias folded into the DVE ysum add against
  a host-replicated bias tile.
- attn@v per head accumulates [65, NQ] in PSUM (row 64 = denominators);
  heads run E(even) then O(odd) per pair with the O ex-stream buffered
  (delayed-O) so a single 2-bank psav suffices; the LAST pair runs both
  heads interleaved with the O head accumulating in the 2 pgen banks
  (free by then) so there is no tail burst.
- Inputs host-packed into 6 dram tensors so the critical path is 2 big DMAs;
  output stored f16 in 2 DMAs and unscrambled on host.
"""

import sys

import numpy as np

if "/opt/trn_rl_repo" not in sys.path:
    sys.path.insert(0, "/opt/trn_rl_repo")

B, N, C = 4, 2048, 512
H, DH = 8, 64
NQ = 1024
P = 128
NCH = 16
VW = DH + 1
MAX_FPS = np.float32(30.0)

_CACHE = {}


def _host_prep(x, mask, times, Wqkv, Wproj, bproj):
    x = np.asarray(x, np.float32)
    mask = np.asarray(mask, np.float32)
    times = np.asarray(times, np.float32)
    Wqkv = np.asarray(Wqkv, np.float32)
    Wproj = np.asarray(Wproj, np.float32)
    bproj = np.asarray(bproj, np.float32)

    wtT = np.ascontiguousarray(Wqkv.T).astype(np.float16)    # [512, 1536]
    wptT = np.ascontiguousarray(Wproj.T).astype(np.float16)  # [512, 512]

    # wt packed [128, 6144]: g0 = per-ci [q-ct0 | k-ct0] (cols 0:1024),
    # g1 = per-ci v (1024:3072), g2 = per-ci [q-ct1:3 | k-ct1:3] (3072:6144)
    w4 = wtT.reshape(4, P, 1536)
    wt_h = np.empty((P, 6144), np.float16)
    for ci in range(4):
        wt_h[:, ci * 256:ci * 256 + 128] = w4[ci][:, 0:128]
        wt_h[:, ci * 256 + 128:ci * 256 + 256] = w4[ci][:, 512:640]
        wt_h[:, 1024 + ci * 512:1024 + (ci + 1) * 512] = w4[ci][:, 1024:1536]
        wt_h[:, 3072 + ci * 768:3072 + ci * 768 + 384] = w4[ci][:, 128:512]
        wt_h[:, 3072 + ci * 768 + 384:3072 + (ci + 1) * 768] = w4[ci][:, 640:1024]

    wpt_h = np.empty((P, 2048), np.float16)  # [hp block][512 out-cols]
    for hp in range(4):
        wpt_h[:, hp * 512:(hp + 1) * 512] = wptT[hp * 128:(hp + 1) * 128, :]

    inv_freq = (np.float32(1.0) /
                (np.float32(10000.0) **
                 (np.arange(0, DH, 2, dtype=np.float32) / np.float32(DH))))
    pos = np.round(times * MAX_FPS)               # [B, N], round-half-even
    ridx = (np.arange(P) % DH) // 2               # row -> pair index
    sgn = np.where(np.arange(P) % 2 == 0, 1.0, -1.0).astype(np.float32)[:, None]

    in_maps = []
    for core in range(8):
        b, qhalf = core // 2, core % 2
        perm = np.arange(N) if qhalf == 0 else np.r_[NQ:N, 0:NQ]
        xb = x[b].T[:, perm]                      # [512, 2048]
        # xt packed [128, (nb, ci, 512)]
        xt_h = np.ascontiguousarray(
            xb.reshape(4, P, 4, 512).transpose(1, 2, 0, 3).reshape(P, 8192)
        ).astype(np.float16)
        freqs = pos[b][perm][None, :] * inv_freq[:, None]   # [32, 2048]
        cos32 = np.cos(freqs)
        sin32 = np.sin(freqs)
        ce_h = np.empty((P, 4096), np.float32)
        ce_h[:, 0:2048] = cos32[ridx]
        ce_h[:, 2048:4096] = sin32[ridx] * sgn    # sign folded for pair-swap
        ce_h = np.ascontiguousarray(ce_h.astype(np.float16))
        em = np.exp(mask[b][perm]).astype(np.float32)
        emask_h = np.ascontiguousarray(em.reshape(NCH, P).T)  # [128, 16]
        bias_h = np.zeros((P, 576), np.float16)
        bias_h[:, 0:512] = bproj[None, :].astype(np.float16)
        bias_h[:, 512:576] = np.float16(1.0)      # ones row for den replicate
        in_maps.append({"xt": xt_h, "wt": wt_h, "wpt": wpt_h, "ce": ce_h,
                        "emask": emask_h, "bias": bias_h})
    return in_maps


def _build_module():
    import concourse.tile as tile
    import concourse.mybir as mybir
    from concourse import bacc

    f32 = mybir.dt.float32
    f16 = mybir.dt.float16
    EXPF = mybir.ActivationFunctionType.Exp
    nc = bacc.Bacc(None, target_bir_lowering=False, debug=False)

    xt_d = nc.dram_tensor("xt", [P, 8192], f16, kind="ExternalInput")
    wt_d = nc.dram_tensor("wt", [P, 6144], f16, kind="ExternalInput")
    wpt_d = nc.dram_tensor("wpt", [P, 2048], f16, kind="ExternalInput")
    ce_d = nc.dram_tensor("ce", [P, 4096], f16, kind="ExternalInput")
    emask_d = nc.dram_tensor("emask", [P, NCH], f32, kind="ExternalInput")
    bias_d = nc.dram_tensor("bias", [P, 576], f16, kind="ExternalInput")
    y_d = nc.dram_tensor("y", [P, 4096], f16, kind="ExternalOutput")

    MM = nc.tensor.matmul
    COPYF = mybir.ActivationFunctionType.Copy
    SWAPM = [i ^ 1 for i in range(32)]

    # virtual-time cost estimates (us), tuned to measured primitives
    EXPT = 1.15      # EXP [128,1024] on ACT
    S_CH = 0.52      # one half-chunk score run (2 MMs + transition)
    AV = 0.52        # attn@v one head-chunk (2 MMs + ldw)
    CHAIN4 = 0.97    # 4-MM K=128 accumulation chain
    DEN2 = 0.52
    OP2 = 0.52
    PCAST, MULF, SHUF, GPADD, ADDF = 0.45, 0.45, 0.70, 1.25, 0.46
    VEVACA = 0.75    # ACT copy [128,512] with per-partition scale
    SCC, RECIP, NMUL = 0.72, 1.25, 1.25

    with tile.TileContext(nc) as tc:
        with (
            tc.tile_pool(name="consts", bufs=1) as consts,
            tc.tile_pool(name="big", bufs=1) as big,
            tc.tile_pool(name="exEp", bufs=10) as exEp,
            tc.tile_pool(name="exOp", bufs=20) as exOp,
            tc.tile_pool(name="tmps", bufs=2) as tmps,
            tc.tile_pool(name="prawp", bufs=2) as prawp,
            tc.tile_pool(name="shfp", bufs=2) as shfp,
            tc.tile_pool(name="scpool", bufs=3) as scpool,
            tc.tile_pool(name="rrp", bufs=2) as rrp,
            tc.tile_pool(name="pss", bufs=2, space="PSUM") as pss,
            tc.tile_pool(name="psavp", bufs=1, space="PSUM") as psavp,
            tc.tile_pool(name="pgen", bufs=2, space="PSUM") as pgen,
        ):
            wt_s = consts.tile([P, 6144], f16, name="wt_s")
            xt_s = consts.tile([P, 4, 4, 512], f16, name="xt_s")
            ce_s = consts.tile([P, 4096], f16, name="ce_s")
            wpt_s = consts.tile([P, 2048], f16, name="wpt_s")
            bias_s = consts.tile([P, 576], f16, name="bias_s")
            emask_s = consts.tile([P, NCH], f32, name="emask_s")

            qT = [big.tile([P, NQ], f16, name=f"qT{i}") for i in range(4)]
            kT = [big.tile([P, N], f16, name=f"kT{i}") for i in range(4)]
            v65 = big.tile([P, NCH, H, VW], f16, name="v65")
            scp = [big.tile([P, NQ], f16, name=f"scp{i}") for i in range(4)]
            ysum = big.tile([P, 8, 512], f16, name="ysum")
            y_all = big.tile([P, 8, 512], f16, name="y_all")

            # ---------------- DMAs (priority order) ----------------
            DMA = nc.sync.dma_start
            DMA(wt_s[:, 0:1024], wt_d.ap()[:, 0:1024])
            DMA(xt_s[:, 0, :, :], xt_d.ap()[:, 0:2048])
            DMA(ce_s[:, 0:1024], ce_d.ap()[:, 0:1024])
            DMA(ce_s[:, 2048:3072], ce_d.ap()[:, 2048:3072])
            DMA(xt_s[:, 1, :, :], xt_d.ap()[:, 2048:4096])
            DMA(emask_s[:], emask_d.ap())
            DMA(bias_s[:], bias_d.ap())
            DMA(wt_s[:, 1024:3072], wt_d.ap()[:, 1024:3072])
            DMA(xt_s[:, 2, :, :], xt_d.ap()[:, 4096:6144])
            DMA(xt_s[:, 3, :, :], xt_d.ap()[:, 6144:8192])
            DMA(ce_s[:, 1024:2048], ce_d.ap()[:, 1024:2048])
            DMA(ce_s[:, 3072:4096], ce_d.ap()[:, 3072:4096])
            DMA(wt_s[:, 3072:6144], wt_d.ap()[:, 3072:6144])
            DMA(wpt_s[:], wpt_d.ap())

            def wt_q(ci, ct):
                o = ci * 256 if ct == 0 else 3072 + ci * 768 + (ct - 1) * 128
                return wt_s[:, o:o + 128]

            def wt_k(ci, ct):
                o = (ci * 256 + 128 if ct == 0
                     else 3072 + ci * 768 + 384 + (ct - 1) * 128)
                return wt_s[:, o:o + 128]

            def wt_v(ci):
                o = 1024 + ci * 512
                return wt_s[:, o:o + 512]

            # ---------------- state ----------------
            st = {"pe": 0.0, "act": 0.0, "dve": 0.0, "gp": 0.0}
            kq_ready = {}
            v_ready = {}
            exp_done = {}
            ex_tiles = {}
            sccopy_done = {}
            norm_done = {}
            den_jobs = []

            # ---------------- emitters ----------------
            def chain_qk(sp, ct, nb, dve_add=False):
                pg = pgen.tile([P, 512], f32, name="pg")
                wf = wt_q if sp == "q" else wt_k
                for ci in range(4):
                    MM(pg[:], wf(ci, ct), xt_s[:, nb, ci, :],
                       start=(ci == 0), stop=(ci == 3))
                st["pe"] += CHAIN4
                tok = nb * 512
                praw = prawp.tile([P, 512], f16, name="praw")
                nc.vector.tensor_copy(praw[:], pg[:])
                t = tmps.tile([P, 2, 512], f16, name="t")
                nc.vector.tensor_mul(t[:, 0, :], praw[:], ce_s[:, tok:tok + 512])
                nc.vector.tensor_mul(t[:, 1, :], praw[:],
                                     ce_s[:, 2048 + tok:2560 + tok])
                ts = shfp.tile([P, 512], f16, name="ts")
                nc.vector.stream_shuffle(ts[:], t[:, 1, :], SWAPM)
                st["dve"] = max(st["dve"], st["pe"]) + PCAST + 2 * MULF + SHUF
                dest = qT[ct] if sp == "q" else kT[ct]
                if dve_add:
                    nc.vector.tensor_add(dest[:, tok:tok + 512], t[:, 0, :], ts[:])
                    st["dve"] += ADDF
                    kq_ready[(sp, ct, nb)] = st["dve"]
                else:
                    nc.gpsimd.tensor_add(dest[:, tok:tok + 512], t[:, 0, :], ts[:])
                    st["gp"] = max(st["gp"], st["dve"]) + GPADD
                    kq_ready[(sp, ct, nb)] = st["gp"]

            def chain_v(c):
                pg = pgen.tile([P, 512], f32, name="pg")
                nb, off = c // 4, (c % 4) * 128
                for ci in range(4):
                    MM(pg[:], xt_s[:, nb, ci, off:off + 128], wt_v(ci),
                       start=(ci == 0), stop=(ci == 3))
                st["pe"] += CHAIN4
                vv = v65[:, c, :, :]
                nc.scalar.activation(
                    vv[:, :, 0:DH],
                    pg[:].rearrange("p (h w) -> p h w", w=DH),
                    COPYF, scale=emask_s[:, c:c + 1])
                st["act"] = max(st["act"], st["pe"]) + VEVACA
                nc.vector.tensor_copy(
                    vv[:, :, DH:DH + 1],
                    emask_s[:, c:c + 1, None].to_broadcast((P, H, 1)))
                st["dve"] = max(st["dve"], st["pe"]) + 0.2
                v_ready[c] = max(st["act"], st["dve"])

            def emit_s_exp(p, c):
                kc = kT[p][:, c * 128:(c + 1) * 128]
                need = max(kq_ready.get(("q", p, 0), 0.0),
                           kq_ready.get(("q", p, 1), 0.0),
                           kq_ready.get(("k", p, c // 4), 0.0))
                st["pe"] = max(st["pe"], need)
                psE = pss.tile([P, NQ], f32, name="ps")
                psO = pss.tile([P, NQ], f32, name="ps")
                for qb in (0, 1):
                    MM(psE[:, qb * 512:(qb + 1) * 512], kc[0:64, :],
                       qT[p][0:64, qb * 512:(qb + 1) * 512],
                       start=True, stop=True)
                st["pe"] += S_CH
                hE, hO = 2 * p, 2 * p + 1
                exE = exEp.tile([P, NQ], f16, name="exE")
                nc.scalar.activation(exE[:], psE[:], EXPF, scale=0.125)
                st["act"] = max(st["act"], st["pe"]) + EXPT
                exp_done[(hE, c)] = st["act"]
                ex_tiles[(hE, c)] = exE
                for qb in (0, 1):
                    MM(psO[:, qb * 512:(qb + 1) * 512], kc[64:128, :],
                       qT[p][64:128, qb * 512:(qb + 1) * 512],
                       start=True, stop=True)
                st["pe"] += S_CH
                exO = exOp.tile([P, NQ], f16, name="exO")
                nc.scalar.activation(exO[:], psO[:], EXPF, scale=0.125)
                st["act"] = max(st["act"], st["pe"] + EXPT, st["act"] + EXPT)
                exp_done[(hO, c)] = st["act"]
                ex_tiles[(hO, c)] = exO

            psav_t = [None]
            pg_o3 = [None, None]

            def emit_av(h, c):
                ex = ex_tiles.pop((h, c))
                if h == 7:
                    if c == 0:
                        pg_o3[0] = pgen.tile([P, 512], f32, name="pg")
                        pg_o3[1] = pgen.tile([P, 512], f32, name="pg")
                    for qb in (0, 1):
                        MM(pg_o3[qb][0:VW, :], v65[:, c, h, :],
                           ex[:, qb * 512:(qb + 1) * 512],
                           start=(c == 0), stop=(c == 15))
                else:
                    if c == 0:
                        psav_t[0] = psavp.tile([VW, NQ], f32, name="ps_o")
                    po = psav_t[0]
                    for qb in (0, 1):
                        MM(po[:, qb * 512:(qb + 1) * 512], v65[:, c, h, :],
                           ex[:, qb * 512:(qb + 1) * 512],
                           start=(c == 0), stop=(c == 15))
                st["pe"] += AV

            def start_epilogue(h):
                sc = scpool.tile([VW, NQ], f16, name="sc")
                if h == 7:
                    for qb in (0, 1):
                        nc.vector.tensor_copy(
                            sc[:, qb * 512:(qb + 1) * 512], pg_o3[qb][0:VW, :])
                    st["dve"] = max(st["dve"], st["pe"]) + SCC
                else:
                    po = psav_t[0]
                    nc.vector.tensor_copy(sc[:], po[:])
                    st["dve"] = max(st["dve"], st["pe"]) + SCC
                sccopy_done[h] = st["dve"]
                den_jobs.append({"h": h, "sc": sc, "ready": st["dve"]})

            def emit_den(job):
                h, sc = job["h"], job["sc"]
                p, off = h // 2, (h % 2) * 64
                st["pe"] = max(st["pe"], job["ready"])
                ones = bias_s[64:65, 512:576]
                rr = rrp.tile([DH, NQ], f32, name="rr")
                if h < 4:
                    pd0 = pgen.tile([P, 512], f32, name="pg")
                    pd1 = pgen.tile([P, 512], f32, name="pg")
                    MM(pd0[0:64, :], ones, sc[64:65, 0:512], start=True, stop=True)
                    MM(pd1[0:64, :], ones, sc[64:65, 512:1024], start=True, stop=True)
                    st["pe"] += DEN2
                    nc.vector.reciprocal_approx_fast(rr[:, 0:512], pd0[0:64, :])
                    nc.vector.reciprocal_approx_fast(rr[:, 512:1024], pd1[0:64, :])
                else:
                    pd = pss.tile([P, NQ], f32, name="ps")
                    MM(pd[0:64, 0:512], ones, sc[64:65, 0:512], start=True, stop=True)
                    MM(pd[0:64, 512:1024], ones, sc[64:65, 512:1024],
                       start=True, stop=True)
                    st["pe"] += DEN2
                    nc.vector.reciprocal_approx_fast(rr[:], pd[0:64, :])
                nc.vector.tensor_mul(scp[p][off:off + 64, :], sc[0:64, :], rr[:])
                st["dve"] = max(st["dve"], st["pe"]) + RECIP + NMUL
                norm_done[h] = st["dve"]

            def emit_opA(nbk):
                pg = pgen.tile([P, 512], f32, name="pg")
                MM(pg[:], scp[0][:, nbk * 128:(nbk + 1) * 128],
                   wpt_s[:, 0:512], start=True, stop=False)
                MM(pg[:], scp[1][:, nbk * 128:(nbk + 1) * 128],
                   wpt_s[:, 512:1024], start=False, stop=True)
                st["pe"] += OP2
                nc.vector.tensor_add(ysum[:, nbk, :], pg[:], bias_s[:, 0:512])
                st["dve"] = max(st["dve"], st["pe"]) + NMUL

            def emit_tail(nbk, sl):
                MM(sl, scp[2][:, nbk * 128:(nbk + 1) * 128],
                   wpt_s[:, 1024:1536], start=True, stop=False)
                MM(sl, scp[3][:, nbk * 128:(nbk + 1) * 128],
                   wpt_s[:, 1536:2048], start=False, stop=True)
                st["pe"] += OP2
                nc.vector.tensor_add(y_all[:, nbk, :], ysum[:, nbk, :], sl)
                st["dve"] = max(st["dve"], st["pe"]) + NMUL

            # ---------------- schedule ----------------
            chain_qk("q", 0, 0, dve_add=True)
            chain_qk("k", 0, 0, dve_add=True)
            chain_qk("q", 0, 1, dve_add=True)

            filler = [("k", 0, 1), ("v", 0), ("k", 0, 2), ("v", 1),
                      ("k", 0, 3), ("v", 2), ("q", 1, 0), ("v", 3),
                      ("q", 1, 1), ("v", 4), ("k", 1, 0), ("v", 5),
                      ("k", 1, 1), ("v", 6), ("k", 1, 2), ("v", 7),
                      ("k", 1, 3), ("v", 8), ("v", 9), ("q", 2, 0),
                      ("v", 10), ("q", 2, 1), ("v", 11), ("k", 2, 0),
                      ("v", 12), ("k", 2, 1), ("v", 13), ("k", 2, 2),
                      ("v", 14), ("k", 2, 3), ("v", 15), ("q", 3, 0),
                      ("q", 3, 1), ("k", 3, 0), ("k", 3, 1), ("k", 3, 2),
                      ("k", 3, 3)]
            fpos = {it: i for i, it in enumerate(filler)}
            fidx = [0]

            def consume_one():
                it = filler[fidx[0]]
                fidx[0] += 1
                if it[0] == "v":
                    chain_v(it[1])
                else:
                    chain_qk(*it)

            def consume_until(i):
                while fidx[0] <= min(i, len(filler) - 1):
                    consume_one()

            def req_s(p, c):
                mx = -1
                for key in (("q", p, 0), ("q", p, 1), ("k", p, c // 4)):
                    if key in fpos:
                        mx = max(mx, fpos[key])
                return mx

            avq = []
            for p in range(3):
                avq += [(2 * p, c) for c in range(NCH)]
                avq += [(2 * p + 1, c) for c in range(NCH)]
            for c in range(NCH):
                avq += [(6, c), (7, c)]
            avi = [0]
            # psav users in order; each gated on predecessor's sccopy
            psav_prev = {0: None, 1: 0, 2: 1, 3: 2, 4: 3, 5: 4, 6: 5}
            opA_q = []
            opA_state = [0]  # 0=locked, 1=unlocked

            def av_ready(slack):
                if avi[0] >= len(avq):
                    return False
                h, c = avq[avi[0]]
                if exp_done.get((h, c), 1e9) > st["pe"] + slack:
                    return False
                if v_ready.get(c, 1e9) > st["pe"] + slack:
                    return False
                if c == 0:
                    if h == 7:
                        # pgen claim: all pgen users must be emitted first
                        if fidx[0] < len(filler) or opA_q or opA_state[0] == 0:
                            return False
                        if any(j["h"] < 4 for j in den_jobs):
                            return False
                    else:
                        prev = psav_prev[h]
                        if prev is not None and \
                                sccopy_done.get(prev, 1e9) > st["pe"] + slack:
                            return False
                return True

            def try_unlock_opA():
                if opA_state[0] == 0 and norm_done.get(3, 1e9) <= st["dve"]:
                    opA_state[0] = 1
                    opA_q.extend(range(8))

            def emit_one_av():
                h, c = avq[avi[0]]
                avi[0] += 1
                emit_av(h, c)
                if c == 15:
                    start_epilogue(h)

            for s in range(64):
                p, c = divmod(s, NCH)
                # paced filler: walk toward the requirement 4 periods out,
                # at most 2 chains per period (avoids bursts that stall EXP)
                tgt = req_s(*divmod(min(s + 4, 63), NCH))
                n = 0
                while fidx[0] <= tgt and n < 2:
                    consume_one()
                    n += 1
                consume_until(req_s(p, c))
                emit_s_exp(p, c)
                while den_jobs and den_jobs[0]["ready"] <= st["pe"] + 0.6:
                    emit_den(den_jobs.pop(0))
                try_unlock_opA()
                # attnv: keep pace with EXP production (2 head-chunks per
                # period) plus one catch-up when the backlog grows
                backlog = 2 * (s + 1) - avi[0]
                nav = 2 + (1 if backlog > 8 else 0)
                navper = 0
                while navper < nav and av_ready(0.3):
                    emit_one_av()
                    navper += 1
                while st["pe"] < st["act"] - 0.1:
                    if av_ready(0.3):
                        emit_one_av()
                    elif den_jobs and den_jobs[0]["ready"] <= st["pe"]:
                        emit_den(den_jobs.pop(0))
                    elif opA_q:
                        emit_opA(opA_q.pop(0))
                        try_unlock_opA()
                    elif fidx[0] < len(filler):
                        consume_one()
                    else:
                        break

            # ---------------- drain ----------------
            guard = 0
            while avi[0] < len(avq):
                guard += 1
                assert guard < 10000, "drain stall"
                try_unlock_opA()
                if av_ready(0.3):
                    emit_one_av()
                elif den_jobs and den_jobs[0]["ready"] <= st["pe"] + 0.3:
                    emit_den(den_jobs.pop(0))
                elif fidx[0] < len(filler):
                    consume_one()
                elif opA_q and opA_state[0] == 1:
                    emit_opA(opA_q.pop(0))
                else:
                    st["pe"] += 0.25  # idle nudge toward blocking dep
            while fidx[0] < len(filler):
                consume_one()
            while den_jobs:
                emit_den(den_jobs.pop(0))
            try_unlock_opA()
            while opA_q:
                emit_opA(opA_q.pop(0))

            for g in range(4):
                pt = pss.tile([P, NQ], f32, name="ps")
                emit_tail(2 * g, pt[:, 0:512])
                emit_tail(2 * g + 1, pt[:, 512:1024])
                if g == 1:
                    DMA(y_d.ap()[:, 0:2048], y_all[:, 0:4, :])
            DMA(y_d.ap()[:, 2048:4096], y_all[:, 4:8, :])

    nc.compile()
    return nc


def _get_module():
    if "nc" not in _CACHE:
        _CACHE["nc"] = _build_module()
    return _CACHE["nc"]


def kernel(x, mask, times, Wqkv, Wproj, bproj, num_cls_token=0, _trace=False):
    from concourse.bass_utils import run_bass_kernel_spmd

    assert int(num_cls_token) == 0, "kernel specialized for num_cls_token=0"
    in_maps = _host_prep(x, mask, times, Wqkv, Wproj, bproj)
    nc = _get_module()
    res = run_bass_kernel_spmd(nc, in_maps, list(range(8)), trace=_trace)
    _CACHE["last_result"] = res

    out = np.empty((B, N, C), np.float32)
    for core in range(8):
        b, qhalf = core // 2, core % 2
        y = np.asarray(res.results[core]["y"], np.float32)   # [128, 4096]
        blk = y.reshape(P, 8, 512).transpose(1, 0, 2).reshape(NQ, C)
        out[b, qhalf * NQ:(qhalf + 1) * NQ, :] = blk
    return out


# revision 42
# speedup vs baseline: 1.0910x; 1.0476x over previous
"""Trainium2 Bass kernel for AttentionWithRotaryPositionalEmbedding (v2).

Shapes (hardcoded): x [4, 2048, 512], 8 heads, head dim 64.
Sharding: 8 cores = (batch b = core//2) x (query half = core%2); k/v computed
locally from the full x[b] so no collectives.

v2 design (measured-primitive driven), ~214-220us vs 241us baseline:
- PE is the pacer (~185us busy: 720 matmuls at the 214ns/MM N=512
  streaming floor + ~90ns per run transition). ACT EXP stream is 143us.
- Scores K=64 matmuls use row-tiling (lhsT/rhs partition bases 0/64);
  the Tile scheduler staggers the E/O halves around the 2-buf pss WAR,
  which is optimal for ACT continuity.
- RoPE rotation via DVE stream_shuffle (partition pair-swap; the sin
  table carries the sign) + GpSimd add, replacing 24 PE matmuls; bias via
  host-replicated tile folded into the DVE ysum add (8 matmuls saved);
  K=1 den-replicate matmuls kept (DMA cannot partition-broadcast).
- attn@v per head accumulates [65, NQ] in PSUM (row 64 = denominators);
  heads run E then O per pair with the O ex-stream buffered (delayed-O,
  exO bufs=24) on a single 2-bank psav; the LAST pair interleaves both
  heads with the O head accumulating in the 2 pgen banks (free by then).
- Den/normalize epilogue for late heads is qb-split (matmul -> recip ->
  mul per 512-col half) so tail outproj matmuls unblock on half ranges.
- 10 garbage warm-up matmuls + a dummy EXP during the DMA lead-in open
  the HAM clock gate (1.2 -> 2.4 GHz) and pull the ACT table load off
  the critical path.
- Inputs host-packed into 6 dram tensors (critical path = 2 split DMAs);
  y stored f16 in 4 progressive DMAs, unscrambled + upcast on host.
- Measured pitfalls: fp8 q/k projection fails the 2e-2 budget (3.8e-2);
  sustained benching thermally throttles the chip ~20% (EXP 1112->1336ns).
"""

import sys

import numpy as np

if "/opt/trn_rl_repo" not in sys.path:
    sys.path.insert(0, "/opt/trn_rl_repo")

B, N, C = 4, 2048, 512
H, DH = 8, 64
NQ = 1024
P = 128
NCH = 16
VW = DH + 1
MAX_FPS = np.float32(30.0)

_CACHE = {}


def _host_prep(x, mask, times, Wqkv, Wproj, bproj):
    x = np.asarray(x, np.float32)
    mask = np.asarray(mask, np.float32)
    times = np.asarray(times, np.float32)
    Wqkv = np.asarray(Wqkv, np.float32)
    Wproj = np.asarray(Wproj, np.float32)
    bproj = np.asarray(bproj, np.float32)

    wtT = np.ascontiguousarray(Wqkv.T).astype(np.float16)    # [512, 1536]
    wptT = np.ascontiguousarray(Wproj.T).astype(np.float16)  # [512, 512]

    # wt packed [128, 6144]: g0 = per-ci [q-ct0 | k-ct0] (cols 0:1024),
    # g1 = per-ci v (1024:3072), g2 = per-ci [q-ct1:3 | k-ct1:3] (3072:6144)
    w4 = wtT.reshape(4, P, 1536)
    wt_h = np.empty((P, 6144), np.float16)
    for ci in range(4):
        wt_h[:, ci * 256:ci * 256 + 128] = w4[ci][:, 0:128]
        wt_h[:, ci * 256 + 128:ci * 256 + 256] = w4[ci][:, 512:640]
        wt_h[:, 1024 + ci * 512:1024 + (ci + 1) * 512] = w4[ci][:, 1024:1536]
        wt_h[:, 3072 + ci * 768:3072 + ci * 768 + 384] = w4[ci][:, 128:512]
        wt_h[:, 3072 + ci * 768 + 384:3072 + (ci + 1) * 768] = w4[ci][:, 640:1024]

    wpt_h = np.empty((P, 2048), np.float16)  # [hp block][512 out-cols]
    for hp in range(4):
        wpt_h[:, hp * 512:(hp + 1) * 512] = wptT[hp * 128:(hp + 1) * 128, :]

    inv_freq = (np.float32(1.0) /
                (np.float32(10000.0) **
                 (np.arange(0, DH, 2, dtype=np.float32) / np.float32(DH))))
    pos = np.round(times * MAX_FPS)               # [B, N], round-half-even
    ridx = (np.arange(P) % DH) // 2               # row -> pair index
    sgn = np.where(np.arange(P) % 2 == 0, 1.0, -1.0).astype(np.float32)[:, None]

    in_maps = []
    for core in range(8):
        b, qhalf = core // 2, core % 2
        perm = np.arange(N) if qhalf == 0 else np.r_[NQ:N, 0:NQ]
        xb = x[b].T[:, perm]                      # [512, 2048]
        # xt packed [128, (nb, ci, 512)]
        xt_h = np.ascontiguousarray(
            xb.reshape(4, P, 4, 512).transpose(1, 2, 0, 3).reshape(P, 8192)
        ).astype(np.float16)
        freqs = pos[b][perm][None, :] * inv_freq[:, None]   # [32, 2048]
        cos32 = np.cos(freqs)
        sin32 = np.sin(freqs)
        ce_h = np.empty((P, 4096), np.float32)
        ce_h[:, 0:2048] = cos32[ridx]
        ce_h[:, 2048:4096] = sin32[ridx] * sgn    # sign folded for pair-swap
        ce_h = np.ascontiguousarray(ce_h.astype(np.float16))
        em = np.exp(mask[b][perm]).astype(np.float32)
        emask_h = np.ascontiguousarray(em.reshape(NCH, P).T)  # [128, 16]
        bias_h = np.zeros((P, 576), np.float16)
        bias_h[:, 0:512] = bproj[None, :].astype(np.float16)
        bias_h[:, 512:576] = np.float16(1.0)      # ones row for den replicate
        in_maps.append({"xt": xt_h, "wt": wt_h, "wpt": wpt_h, "ce": ce_h,
                        "emask": emask_h, "bias": bias_h})
    return in_maps


def _build_module():
    import concourse.tile as tile
    import concourse.mybir as mybir
    from concourse import bacc

    f32 = mybir.dt.float32
    f16 = mybir.dt.float16
    EXPF = mybir.ActivationFunctionType.Exp
    nc = bacc.Bacc(None, target_bir_lowering=False, debug=False)

    xt_d = nc.dram_tensor("xt", [P, 8192], f16, kind="ExternalInput")
    wt_d = nc.dram_tensor("wt", [P, 6144], f16, kind="ExternalInput")
    wpt_d = nc.dram_tensor("wpt", [P, 2048], f16, kind="ExternalInput")
    ce_d = nc.dram_tensor("ce", [P, 4096], f16, kind="ExternalInput")
    emask_d = nc.dram_tensor("emask", [P, NCH], f32, kind="ExternalInput")
    bias_d = nc.dram_tensor("bias", [P, 576], f16, kind="ExternalInput")
    y_d = nc.dram_tensor("y", [P, 4096], f16, kind="ExternalOutput")

    MM = nc.tensor.matmul
    COPYF = mybir.ActivationFunctionType.Copy
    SWAPM = [i ^ 1 for i in range(32)]

    # virtual-time cost estimates (us), tuned to measured primitives
    EXPT = 1.15      # EXP [128,1024] on ACT
    S_CH = 0.52      # one half-chunk score run (2 MMs + transition)
    AV = 0.52        # attn@v one head-chunk (2 MMs + ldw)
    CHAIN4 = 0.97    # 4-MM K=128 accumulation chain
    DEN2 = 0.52
    OP2 = 0.52
    PCAST, MULF, SHUF, GPADD, ADDF = 0.45, 0.45, 0.70, 1.25, 0.46
    VEVACA = 0.75    # ACT copy [128,512] with per-partition scale
    SCC, RECIP, NMUL = 0.72, 1.25, 1.25

    with tile.TileContext(nc) as tc:
        with (
            tc.tile_pool(name="consts", bufs=1) as consts,
            tc.tile_pool(name="big", bufs=1) as big,
            tc.tile_pool(name="exEp", bufs=12) as exEp,
            tc.tile_pool(name="exOp", bufs=24) as exOp,
            tc.tile_pool(name="tmps", bufs=2) as tmps,
            tc.tile_pool(name="prawp", bufs=2) as prawp,
            tc.tile_pool(name="shfp", bufs=2) as shfp,
            tc.tile_pool(name="scpool", bufs=3) as scpool,
            tc.tile_pool(name="rrp", bufs=2) as rrp,
            tc.tile_pool(name="pss", bufs=2, space="PSUM") as pss,
            tc.tile_pool(name="psavp", bufs=1, space="PSUM") as psavp,
            tc.tile_pool(name="pgen", bufs=2, space="PSUM") as pgen,
        ):
            wt_s = consts.tile([P, 6144], f16, name="wt_s")
            xt_s = consts.tile([P, 4, 4, 512], f16, name="xt_s")
            ce_s = consts.tile([P, 4096], f16, name="ce_s")
            wpt_s = consts.tile([P, 2048], f16, name="wpt_s")
            bias_s = consts.tile([P, 576], f16, name="bias_s")
            emask_s = consts.tile([P, NCH], f32, name="emask_s")

            qT = [big.tile([P, NQ], f16, name=f"qT{i}") for i in range(4)]
            kT = [big.tile([P, N], f16, name=f"kT{i}") for i in range(4)]
            v65 = big.tile([P, NCH, H, VW], f16, name="v65")
            scp = [big.tile([P, NQ], f16, name=f"scp{i}") for i in range(4)]
            ysum = big.tile([P, 8, 512], f16, name="ysum")
            y_all = big.tile([P, 8, 512], f16, name="y_all")

            # ---------------- DMAs (priority order) ----------------
            # first chain (q ct0 nb0) consumes ci blocks in order, so split
            # the critical slabs in halves to let its first MMs start early
            DMA = nc.sync.dma_start
            DMA(wt_s[:, 0:512], wt_d.ap()[:, 0:512])
            DMA(xt_s[:, 0, 0:2, :], xt_d.ap()[:, 0:1024])
            DMA(wt_s[:, 512:1024], wt_d.ap()[:, 512:1024])
            DMA(xt_s[:, 0, 2:4, :], xt_d.ap()[:, 1024:2048])
            DMA(ce_s[:, 0:1024], ce_d.ap()[:, 0:1024])
            DMA(ce_s[:, 2048:3072], ce_d.ap()[:, 2048:3072])
            DMA(xt_s[:, 1, :, :], xt_d.ap()[:, 2048:4096])
            DMA(emask_s[:], emask_d.ap())
            DMA(bias_s[:], bias_d.ap())
            DMA(wt_s[:, 1024:3072], wt_d.ap()[:, 1024:3072])
            DMA(xt_s[:, 2, :, :], xt_d.ap()[:, 4096:6144])
            DMA(xt_s[:, 3, :, :], xt_d.ap()[:, 6144:8192])
            DMA(ce_s[:, 1024:2048], ce_d.ap()[:, 1024:2048])
            DMA(ce_s[:, 3072:4096], ce_d.ap()[:, 3072:4096])
            DMA(wt_s[:, 3072:6144], wt_d.ap()[:, 3072:6144])
            DMA(wpt_s[:], wpt_d.ap())

            def wt_q(ci, ct):
                o = ci * 256 if ct == 0 else 3072 + ci * 768 + (ct - 1) * 128
                return wt_s[:, o:o + 128]

            def wt_k(ci, ct):
                o = (ci * 256 + 128 if ct == 0
                     else 3072 + ci * 768 + 384 + (ct - 1) * 128)
                return wt_s[:, o:o + 128]

            def wt_v(ci):
                o = 1024 + ci * 512
                return wt_s[:, o:o + 512]

            # ---------------- state ----------------
            st = {"pe": 0.0, "act": 0.0, "dve": 0.0, "gp": 0.0}
            kq_ready = {}
            v_ready = {}
            exp_done = {}
            ex_tiles = {}
            sccopy_done = {}
            norm_done = {}
            den_jobs = []

            # ---------------- emitters ----------------
            qk_count = [0]

            def chain_qk(sp, ct, nb, dve_add=False):
                pg = pgen.tile([P, 512], f32, name="pg")
                wf = wt_q if sp == "q" else wt_k
                for ci in range(4):
                    MM(pg[:], wf(ci, ct), xt_s[:, nb, ci, :],
                       start=(ci == 0), stop=(ci == 3))
                st["pe"] += CHAIN4
                tok = nb * 512
                praw = prawp.tile([P, 512], f16, name="praw")
                qk_count[0] += 1
                if qk_count[0] <= 0:
                    # early phase: ACT is idle and DVE is the crunch
                    nc.scalar.activation(praw[:], pg[:], COPYF)
                    st["act"] = max(st["act"], st["pe"]) + VEVACA
                    st["dve"] = max(st["dve"], st["act"])
                else:
                    nc.vector.tensor_copy(praw[:], pg[:])
                t = tmps.tile([P, 2, 512], f16, name="t")
                nc.vector.tensor_mul(t[:, 0, :], praw[:], ce_s[:, tok:tok + 512])
                nc.vector.tensor_mul(t[:, 1, :], praw[:],
                                     ce_s[:, 2048 + tok:2560 + tok])
                ts = shfp.tile([P, 512], f16, name="ts")
                nc.vector.stream_shuffle(ts[:], t[:, 1, :], SWAPM)
                st["dve"] = max(st["dve"], st["pe"]) + PCAST + 2 * MULF + SHUF
                dest = qT[ct] if sp == "q" else kT[ct]
                if dve_add:
                    nc.vector.tensor_add(dest[:, tok:tok + 512], t[:, 0, :], ts[:])
                    st["dve"] += ADDF
                    kq_ready[(sp, ct, nb)] = st["dve"]
                else:
                    nc.gpsimd.tensor_add(dest[:, tok:tok + 512], t[:, 0, :], ts[:])
                    st["gp"] = max(st["gp"], st["dve"]) + GPADD
                    kq_ready[(sp, ct, nb)] = st["gp"]

            def chain_v(c):
                pg = pgen.tile([P, 512], f32, name="pg")
                nb, off = c // 4, (c % 4) * 128
                for ci in range(4):
                    MM(pg[:], xt_s[:, nb, ci, off:off + 128], wt_v(ci),
                       start=(ci == 0), stop=(ci == 3))
                st["pe"] += CHAIN4
                vv = v65[:, c, :, :]
                nc.scalar.activation(
                    vv[:, :, 0:DH],
                    pg[:].rearrange("p (h w) -> p h w", w=DH),
                    COPYF, scale=emask_s[:, c:c + 1])
                st["act"] = max(st["act"], st["pe"]) + VEVACA
                nc.vector.tensor_copy(
                    vv[:, :, DH:DH + 1],
                    emask_s[:, c:c + 1, None].to_broadcast((P, H, 1)))
                st["dve"] = max(st["dve"], st["pe"]) + 0.2
                v_ready[c] = max(st["act"], st["dve"])

            def s_war(p, c):
                hE = 2 * p
                return max(kq_ready.get(("q", p, 0), 0.0),
                           kq_ready.get(("q", p, 1), 0.0),
                           kq_ready.get(("k", p, c // 4), 0.0),
                           exp_done.get((hE, c - 1), 0.0))

            def emit_s_exp(p, c):
                kc = kT[p][:, c * 128:(c + 1) * 128]
                st["pe"] = max(st["pe"], s_war(p, c) - 0.45)
                psE = pss.tile([P, NQ], f32, name="ps")
                psO = pss.tile([P, NQ], f32, name="ps")
                for qb in (0, 1):
                    MM(psE[:, qb * 512:(qb + 1) * 512], kc[0:64, :],
                       qT[p][0:64, qb * 512:(qb + 1) * 512],
                       start=True, stop=True)
                st["pe"] += S_CH
                hE, hO = 2 * p, 2 * p + 1
                exE = exEp.tile([P, NQ], f16, name="exE")
                nc.scalar.activation(exE[:], psE[:], EXPF, scale=0.125)
                st["act"] = max(st["act"], st["pe"]) + EXPT
                exp_done[(hE, c)] = st["act"]
                ex_tiles[(hE, c)] = exE
                for qb in (0, 1):
                    MM(psO[:, qb * 512:(qb + 1) * 512], kc[64:128, :],
                       qT[p][64:128, qb * 512:(qb + 1) * 512],
                       start=True, stop=True)
                st["pe"] += S_CH
                exO = exOp.tile([P, NQ], f16, name="exO")
                nc.scalar.activation(exO[:], psO[:], EXPF, scale=0.125)
                st["act"] = max(st["act"], st["pe"] + EXPT, st["act"] + EXPT)
                exp_done[(hO, c)] = st["act"]
                ex_tiles[(hO, c)] = exO

            psav_t = [None]
            pg_o3 = [None, None]

            def emit_av(h, c):
                ex = ex_tiles.pop((h, c))
                if h == 7:
                    if c == 0:
                        pg_o3[0] = pgen.tile([P, 512], f32, name="pg")
                        pg_o3[1] = pgen.tile([P, 512], f32, name="pg")
                    for qb in (0, 1):
                        MM(pg_o3[qb][0:VW, :], v65[:, c, h, :],
                           ex[:, qb * 512:(qb + 1) * 512],
                           start=(c == 0), stop=(c == 15))
                else:
                    if c == 0:
                        psav_t[0] = psavp.tile([VW, NQ], f32, name="ps_o")
                    po = psav_t[0]
                    for qb in (0, 1):
                        MM(po[:, qb * 512:(qb + 1) * 512], v65[:, c, h, :],
                           ex[:, qb * 512:(qb + 1) * 512],
                           start=(c == 0), stop=(c == 15))
                st["pe"] += AV

            def start_epilogue(h):
                sc = scpool.tile([VW, NQ], f16, name="sc")
                if h == 7:
                    for qb in (0, 1):
                        nc.vector.tensor_copy(
                            sc[:, qb * 512:(qb + 1) * 512], pg_o3[qb][0:VW, :])
                    st["dve"] = max(st["dve"], st["pe"]) + SCC
                else:
                    po = psav_t[0]
                    nc.vector.tensor_copy(sc[:], po[:])
                    st["dve"] = max(st["dve"], st["pe"]) + SCC
                sccopy_done[h] = st["dve"]
                den_jobs.append({"h": h, "sc": sc, "ready": st["dve"]})

            def emit_den(job):
                h, sc = job["h"], job["sc"]
                p, off = h // 2, (h % 2) * 64
                st["pe"] = max(st["pe"], job["ready"])
                ones = bias_s[64:65, 512:576]
                rr = rrp.tile([DH, NQ], f32, name="rr")
                if h < 4:
                    pd0 = pgen.tile([P, 512], f32, name="pg")
                    pd1 = pgen.tile([P, 512], f32, name="pg")
                    MM(pd0[0:64, :], ones, sc[64:65, 0:512], start=True, stop=True)
                    MM(pd1[0:64, :], ones, sc[64:65, 512:1024], start=True, stop=True)
                    st["pe"] += DEN2
                    nc.vector.reciprocal_approx_fast(rr[:, 0:512], pd0[0:64, :])
                    nc.vector.reciprocal_approx_fast(rr[:, 512:1024], pd1[0:64, :])
                else:
                    # qb-pipelined so late consumers unblock on half ranges
                    pd = pss.tile([P, NQ], f32, name="ps")
                    for qb in (0, 1):
                        sl = slice(qb * 512, (qb + 1) * 512)
                        MM(pd[0:64, sl], ones, sc[64:65, sl], start=True, stop=True)
                        st["pe"] += DEN2 / 2
                        nc.vector.reciprocal_approx_fast(rr[:, sl], pd[0:64, sl])
                        nc.vector.tensor_mul(scp[p][off:off + 64, sl],
                                             sc[0:64, sl], rr[:, sl])
                    st["dve"] = max(st["dve"], st["pe"]) + RECIP + NMUL
                    norm_done[h] = st["dve"]
                    return
                nc.vector.tensor_mul(scp[p][off:off + 64, :], sc[0:64, :], rr[:])
                st["dve"] = max(st["dve"], st["pe"]) + RECIP + NMUL
                norm_done[h] = st["dve"]

            def emit_opA(nbk):
                pg = pgen.tile([P, 512], f32, name="pg")
                MM(pg[:], scp[0][:, nbk * 128:(nbk + 1) * 128],
                   wpt_s[:, 0:512], start=True, stop=False)
                MM(pg[:], scp[1][:, nbk * 128:(nbk + 1) * 128],
                   wpt_s[:, 512:1024], start=False, stop=True)
                st["pe"] += OP2
                nc.vector.tensor_add(ysum[:, nbk, :], pg[:], bias_s[:, 0:512])
                st["dve"] = max(st["dve"], st["pe"]) + NMUL

            def emit_tail(nbk, sl):
                MM(sl, scp[2][:, nbk * 128:(nbk + 1) * 128],
                   wpt_s[:, 1024:1536], start=True, stop=False)
                MM(sl, scp[3][:, nbk * 128:(nbk + 1) * 128],
                   wpt_s[:, 1536:2048], start=False, stop=True)
                st["pe"] += OP2
                nc.vector.tensor_add(y_all[:, nbk, :], ysum[:, nbk, :], sl)
                st["dve"] = max(st["dve"], st["pe"]) + NMUL

            # ---------------- schedule ----------------
            # HAM warm-up: dummy matmuls on garbage SBUF during the DMA
            # lead-in so the PE clock gate opens (1.2 -> 2.4 GHz) before the
            # first real chain; plus a dummy ACTIVATE to pull the ~2.7us
            # ACT table load off the critical path.
            warm = tmps.tile([P, 2, 512], f16, name="t")
            wps = pss.tile([P, NQ], f32, name="ps")
            for i in range(10):
                MM(wps[:, 0:512], warm[:, 0, 0:128], warm[:, 1, :],
                   start=True, stop=True)
            nc.scalar.activation(warm[:, 0, 0:16], wps[:, 0:16], EXPF,
                                 scale=0.125)
            chain_qk("q", 0, 0, dve_add=True)
            chain_qk("k", 0, 0, dve_add=True)
            chain_qk("q", 0, 1, dve_add=True)

            filler = [("k", 0, 1), ("v", 0), ("k", 0, 2), ("v", 1),
                      ("k", 0, 3), ("v", 2), ("q", 1, 0), ("v", 3),
                      ("q", 1, 1), ("v", 4), ("k", 1, 0), ("v", 5),
                      ("k", 1, 1), ("v", 6), ("k", 1, 2), ("v", 7),
                      ("k", 1, 3), ("v", 8), ("v", 9), ("q", 2, 0),
                      ("v", 10), ("q", 2, 1), ("v", 11), ("k", 2, 0),
                      ("v", 12), ("k", 2, 1), ("v", 13), ("k", 2, 2),
                      ("v", 14), ("k", 2, 3), ("v", 15), ("q", 3, 0),
                      ("q", 3, 1), ("k", 3, 0), ("k", 3, 1), ("k", 3, 2),
                      ("k", 3, 3)]
            fpos = {it: i for i, it in enumerate(filler)}
            fidx = [0]

            def consume_one():
                it = filler[fidx[0]]
                fidx[0] += 1
                if it[0] == "v":
                    chain_v(it[1])
                else:
                    chain_qk(*it)

            def consume_until(i):
                while fidx[0] <= min(i, len(filler) - 1):
                    consume_one()

            def req_s(p, c):
                mx = -1
                for key in (("q", p, 0), ("q", p, 1), ("k", p, c // 4)):
                    if key in fpos:
                        mx = max(mx, fpos[key])
                return mx

            avq = []
            for p in range(3):
                avq += [(2 * p, c) for c in range(NCH)]
                avq += [(2 * p + 1, c) for c in range(NCH)]
            for c in range(NCH):
                avq += [(6, c), (7, c)]
            avi = [0]
            # psav users in order; each gated on predecessor's sccopy
            psav_prev = {0: None, 1: 0, 2: 1, 3: 2, 4: 3, 5: 4, 6: 5}
            opA_q = []
            opA_state = [0]  # 0=locked, 1=unlocked

            def av_ready(slack):
                if avi[0] >= len(avq):
                    return False
                h, c = avq[avi[0]]
                if exp_done.get((h, c), 1e9) > st["pe"] + slack:
                    return False
                if v_ready.get(c, 1e9) > st["pe"] + slack:
                    return False
                if c == 0:
                    if h == 7:
                        # pgen claim: all pgen users must be emitted first
                        if fidx[0] < len(filler) or opA_q or opA_state[0] == 0:
                            return False
                        if any(j["h"] < 4 for j in den_jobs):
                            return False
                    else:
                        prev = psav_prev[h]
                        if prev is not None and \
                                sccopy_done.get(prev, 1e9) > st["pe"] + slack:
                            return False
                return True

            def try_unlock_opA():
                if opA_state[0] == 0 and norm_done.get(3, 1e9) <= st["dve"]:
                    opA_state[0] = 1
                    opA_q.extend(range(8))

            def emit_one_av():
                h, c = avq[avi[0]]
                avi[0] += 1
                emit_av(h, c)
                if c == 15:
                    start_epilogue(h)

            for s in range(64):
                p, c = divmod(s, NCH)
                # paced filler: walk toward the requirement 4 periods out,
                # at most 2 chains per period (avoids bursts that stall EXP)
                if s >= 2:
                    tgt = req_s(*divmod(min(s + 4, 63), NCH))
                    n = 0
                    while fidx[0] <= tgt and n < 2:
                        consume_one()
                        n += 1
                consume_until(req_s(p, c))
                emit_s_exp(p, c)
                while den_jobs and den_jobs[0]["ready"] <= st["pe"] + 0.6:
                    emit_den(den_jobs.pop(0))
                try_unlock_opA()
                # attnv: keep pace with EXP production (2 head-chunks per
                # period) plus one catch-up when the backlog grows
                backlog = 2 * (s + 1) - avi[0]
                nav = 2 + (1 if backlog > 8 else 0) + (1 if backlog > 16 else 0)
                navper = 0
                while navper < nav and av_ready(0.3):
                    emit_one_av()
                    navper += 1
                while st["pe"] < st["act"] - 0.1:
                    if av_ready(0.3):
                        emit_one_av()
                    elif den_jobs and den_jobs[0]["ready"] <= st["pe"]:
                        emit_den(den_jobs.pop(0))
                    elif opA_q:
                        emit_opA(opA_q.pop(0))
                        try_unlock_opA()
                    elif fidx[0] < len(filler):
                        consume_one()
                    else:
                        break

            # ---------------- drain ----------------
            guard = 0
            while avi[0] < len(avq):
                guard += 1
                assert guard < 10000, "drain stall"
                try_unlock_opA()
                if av_ready(0.3):
                    emit_one_av()
                elif den_jobs and den_jobs[0]["ready"] <= st["pe"] + 0.3:
                    emit_den(den_jobs.pop(0))
                elif fidx[0] < len(filler):
                    consume_one()
                elif opA_q and opA_state[0] == 1:
                    emit_opA(opA_q.pop(0))
                else:
                    st["pe"] += 0.25  # idle nudge toward blocking dep
            while fidx[0] < len(filler):
                consume_one()
            while den_jobs:
                emit_den(den_jobs.pop(0))
            try_unlock_opA()
            while opA_q:
                emit_opA(opA_q.pop(0))

            for g in range(4):
                pt = pss.tile([P, NQ], f32, name="ps")
                emit_tail(2 * g, pt[:, 0:512])
                emit_tail(2 * g + 1, pt[:, 512:1024])
                DMA(y_d.ap()[:, g * 1024:(g + 1) * 1024],
                    y_all[:, 2 * g:2 * g + 2, :])

    nc.compile()
    return nc


def _get_module():
    if "nc" not in _CACHE:
        _CACHE["nc"] = _build_module()
    return _CACHE["nc"]


def kernel(x, mask, times, Wqkv, Wproj, bproj, num_cls_token=0, _trace=False):
    from concourse.bass_utils import run_bass_kernel_spmd

    assert int(num_cls_token) == 0, "kernel specialized for num_cls_token=0"
    in_maps = _host_prep(x, mask, times, Wqkv, Wproj, bproj)
    nc = _get_module()
    res = run_bass_kernel_spmd(nc, in_maps, list(range(8)), trace=_trace)
    _CACHE["last_result"] = res

    out = np.empty((B, N, C), np.float32)
    for core in range(8):
        b, qhalf = core // 2, core % 2
        y = np.asarray(res.results[core]["y"], np.float32)   # [128, 4096]
        blk = y.reshape(P, 8, 512).transpose(1, 0, 2).reshape(NQ, C)
        out[b, qhalf * NQ:(qhalf + 1) * NQ, :] = blk
    return out


# revision 45
# speedup vs baseline: 1.0920x; 1.0010x over previous
"""Trainium2 Bass kernel for AttentionWithRotaryPositionalEmbedding (v2).

Shapes (hardcoded): x [4, 2048, 512], 8 heads, head dim 64.
Sharding: 8 cores = (batch b = core//2) x (query half = core%2); k/v computed
locally from the full x[b] so no collectives.

v2 design (measured-primitive driven), ~214-220us vs 241us baseline:
- PE is the pacer (~185us busy: 720 matmuls at the 214ns/MM N=512
  streaming floor + ~90ns per run transition). ACT EXP stream is 143us.
- Scores K=64 matmuls use row-tiling (lhsT/rhs partition bases 0/64);
  the Tile scheduler staggers the E/O halves around the 2-buf pss WAR,
  which is optimal for ACT continuity.
- RoPE rotation via DVE stream_shuffle (partition pair-swap; the sin
  table carries the sign) + GpSimd add, replacing 24 PE matmuls; bias via
  host-replicated tile folded into the DVE ysum add (8 matmuls saved);
  K=1 den-replicate matmuls kept (DMA cannot partition-broadcast).
- attn@v per head accumulates [65, NQ] in PSUM (row 64 = denominators);
  heads run E then O per pair with the O ex-stream buffered (delayed-O,
  exO bufs=24) on a single 2-bank psav; the LAST pair interleaves both
  heads with the O head accumulating in the 2 pgen banks (free by then).
- Den/normalize epilogue for late heads is qb-split (matmul -> recip ->
  mul per 512-col half) so tail outproj matmuls unblock on half ranges.
- 10 garbage warm-up matmuls + a dummy EXP during the DMA lead-in open
  the HAM clock gate (1.2 -> 2.4 GHz) and pull the ACT table load off
  the critical path.
- Inputs host-packed into 6 dram tensors (critical path = 2 split DMAs);
  y stored f16 in 4 progressive DMAs, unscrambled + upcast on host.
- Measured pitfalls: fp8 q/k projection fails the 2e-2 budget (3.8e-2);
  sustained benching thermally throttles the chip ~20% (EXP 1112->1336ns).
"""

import sys

import numpy as np

if "/opt/trn_rl_repo" not in sys.path:
    sys.path.insert(0, "/opt/trn_rl_repo")

B, N, C = 4, 2048, 512
H, DH = 8, 64
NQ = 1024
P = 128
NCH = 16
VW = DH + 1
MAX_FPS = np.float32(30.0)

_CACHE = {}


def _host_prep(x, mask, times, Wqkv, Wproj, bproj):
    x = np.asarray(x, np.float32)
    mask = np.asarray(mask, np.float32)
    times = np.asarray(times, np.float32)
    Wqkv = np.asarray(Wqkv, np.float32)
    Wproj = np.asarray(Wproj, np.float32)
    bproj = np.asarray(bproj, np.float32)

    wtT = np.ascontiguousarray(Wqkv.T).astype(np.float16)    # [512, 1536]
    wptT = np.ascontiguousarray(Wproj.T).astype(np.float16)  # [512, 512]

    # wt packed [128, 6144]: g0 = per-ci [q-ct0 | k-ct0] (cols 0:1024),
    # g1 = per-ci v (1024:3072), g2 = per-ci [q-ct1:3 | k-ct1:3] (3072:6144)
    w4 = wtT.reshape(4, P, 1536)
    wt_h = np.empty((P, 6144), np.float16)
    for ci in range(4):
        wt_h[:, ci * 256:ci * 256 + 128] = w4[ci][:, 0:128]
        wt_h[:, ci * 256 + 128:ci * 256 + 256] = w4[ci][:, 512:640]
        wt_h[:, 1024 + ci * 512:1024 + (ci + 1) * 512] = w4[ci][:, 1024:1536]
        wt_h[:, 3072 + ci * 768:3072 + ci * 768 + 384] = w4[ci][:, 128:512]
        wt_h[:, 3072 + ci * 768 + 384:3072 + (ci + 1) * 768] = w4[ci][:, 640:1024]

    wpt_h = np.empty((P, 2048), np.float16)  # [hp block][512 out-cols]
    for hp in range(4):
        wpt_h[:, hp * 512:(hp + 1) * 512] = wptT[hp * 128:(hp + 1) * 128, :]

    inv_freq = (np.float32(1.0) /
                (np.float32(10000.0) **
                 (np.arange(0, DH, 2, dtype=np.float32) / np.float32(DH))))
    pos = np.round(times * MAX_FPS)               # [B, N], round-half-even
    ridx = (np.arange(P) % DH) // 2               # row -> pair index
    sgn = np.where(np.arange(P) % 2 == 0, 1.0, -1.0).astype(np.float32)[:, None]

    in_maps = []
    for core in range(8):
        b, qhalf = core // 2, core % 2
        perm = np.arange(N) if qhalf == 0 else np.r_[NQ:N, 0:NQ]
        xb = x[b].T[:, perm]                      # [512, 2048]
        # xt packed [128, (nb, ci, 512)]
        xt_h = np.ascontiguousarray(
            xb.reshape(4, P, 4, 512).transpose(1, 2, 0, 3).reshape(P, 8192)
        ).astype(np.float16)
        freqs = pos[b][perm][None, :] * inv_freq[:, None]   # [32, 2048]
        cos32 = np.cos(freqs)
        sin32 = np.sin(freqs)
        ce_h = np.empty((P, 4096), np.float32)
        ce_h[:, 0:2048] = cos32[ridx]
        ce_h[:, 2048:4096] = sin32[ridx] * sgn    # sign folded for pair-swap
        ce_h = np.ascontiguousarray(ce_h.astype(np.float16))
        em = np.exp(mask[b][perm]).astype(np.float32)
        emask_h = np.ascontiguousarray(em.reshape(NCH, P).T)  # [128, 16]
        bias_h = np.zeros((P, 576), np.float16)
        bias_h[:, 0:512] = bproj[None, :].astype(np.float16)
        bias_h[:, 512:576] = np.float16(1.0)      # ones row for den replicate
        in_maps.append({"xt": xt_h, "wt": wt_h, "wpt": wpt_h, "ce": ce_h,
                        "emask": emask_h, "bias": bias_h})
    return in_maps


def _build_module():
    import concourse.tile as tile
    import concourse.mybir as mybir
    from concourse import bacc

    f32 = mybir.dt.float32
    f16 = mybir.dt.float16
    EXPF = mybir.ActivationFunctionType.Exp
    nc = bacc.Bacc(None, target_bir_lowering=False, debug=False)

    xt_d = nc.dram_tensor("xt", [P, 8192], f16, kind="ExternalInput")
    wt_d = nc.dram_tensor("wt", [P, 6144], f16, kind="ExternalInput")
    wpt_d = nc.dram_tensor("wpt", [P, 2048], f16, kind="ExternalInput")
    ce_d = nc.dram_tensor("ce", [P, 4096], f16, kind="ExternalInput")
    emask_d = nc.dram_tensor("emask", [P, NCH], f32, kind="ExternalInput")
    bias_d = nc.dram_tensor("bias", [P, 576], f16, kind="ExternalInput")
    y_d = nc.dram_tensor("y", [P, 4096], f16, kind="ExternalOutput")

    MM = nc.tensor.matmul
    COPYF = mybir.ActivationFunctionType.Copy
    SWAPM = [i ^ 1 for i in range(32)]

    # virtual-time cost estimates (us), tuned to measured primitives
    EXPT = 1.15      # EXP [128,1024] on ACT
    S_CH = 0.52      # one half-chunk score run (2 MMs + transition)
    AV = 0.52        # attn@v one head-chunk (2 MMs + ldw)
    CHAIN4 = 0.97    # 4-MM K=128 accumulation chain
    DEN2 = 0.52
    OP2 = 0.52
    PCAST, MULF, SHUF, GPADD, ADDF = 0.45, 0.45, 0.70, 1.25, 0.46
    VEVACA = 0.75    # ACT copy [128,512] with per-partition scale
    SCC, RECIP, NMUL = 0.72, 1.25, 1.25

    with tile.TileContext(nc) as tc:
        with (
            tc.tile_pool(name="consts", bufs=1) as consts,
            tc.tile_pool(name="big", bufs=1) as big,
            tc.tile_pool(name="exEp", bufs=12) as exEp,
            tc.tile_pool(name="exOp", bufs=24) as exOp,
            tc.tile_pool(name="tmps", bufs=2) as tmps,
            tc.tile_pool(name="prawp", bufs=2) as prawp,
            tc.tile_pool(name="shfp", bufs=2) as shfp,
            tc.tile_pool(name="scpool", bufs=3) as scpool,
            tc.tile_pool(name="rrp", bufs=2) as rrp,
            tc.tile_pool(name="pss", bufs=2, space="PSUM") as pss,
            tc.tile_pool(name="psavp", bufs=1, space="PSUM") as psavp,
            tc.tile_pool(name="pgen", bufs=2, space="PSUM") as pgen,
        ):
            wt_s = consts.tile([P, 6144], f16, name="wt_s")
            xt_s = consts.tile([P, 4, 4, 512], f16, name="xt_s")
            ce_s = consts.tile([P, 4096], f16, name="ce_s")
            wpt_s = consts.tile([P, 2048], f16, name="wpt_s")
            bias_s = consts.tile([P, 576], f16, name="bias_s")
            emask_s = consts.tile([P, NCH], f32, name="emask_s")

            gb = big.tile([P, 2, 512], f16, name="gb")  # garbage, dummy MMs
            qT = [big.tile([P, NQ], f16, name=f"qT{i}") for i in range(4)]
            kT = [big.tile([P, N], f16, name=f"kT{i}") for i in range(4)]
            v65 = big.tile([P, NCH, H, VW], f16, name="v65")
            scp = [big.tile([P, NQ], f16, name=f"scp{i}") for i in range(4)]
            ysum = big.tile([P, 8, 512], f16, name="ysum")
            y_all = big.tile([P, 8, 512], f16, name="y_all")

            # ---------------- DMAs (priority order) ----------------
            # first chain (q ct0 nb0) consumes ci blocks in order, so split
            # the critical slabs in halves to let its first MMs start early
            DMA = nc.sync.dma_start
            DMA(wt_s[:, 0:512], wt_d.ap()[:, 0:512])
            DMA(xt_s[:, 0, 0:2, :], xt_d.ap()[:, 0:1024])
            DMA(wt_s[:, 512:1024], wt_d.ap()[:, 512:1024])
            DMA(xt_s[:, 0, 2:4, :], xt_d.ap()[:, 1024:2048])
            DMA(ce_s[:, 0:1024], ce_d.ap()[:, 0:1024])
            DMA(ce_s[:, 2048:3072], ce_d.ap()[:, 2048:3072])
            DMA(xt_s[:, 1, :, :], xt_d.ap()[:, 2048:4096])
            DMA(emask_s[:], emask_d.ap())
            DMA(bias_s[:], bias_d.ap())
            DMA(wt_s[:, 1024:3072], wt_d.ap()[:, 1024:3072])
            DMA(xt_s[:, 2, :, :], xt_d.ap()[:, 4096:6144])
            DMA(xt_s[:, 3, :, :], xt_d.ap()[:, 6144:8192])
            DMA(ce_s[:, 1024:2048], ce_d.ap()[:, 1024:2048])
            DMA(ce_s[:, 3072:4096], ce_d.ap()[:, 3072:4096])
            DMA(wt_s[:, 3072:6144], wt_d.ap()[:, 3072:6144])
            DMA(wpt_s[:], wpt_d.ap())

            def wt_q(ci, ct):
                o = ci * 256 if ct == 0 else 3072 + ci * 768 + (ct - 1) * 128
                return wt_s[:, o:o + 128]

            def wt_k(ci, ct):
                o = (ci * 256 + 128 if ct == 0
                     else 3072 + ci * 768 + 384 + (ct - 1) * 128)
                return wt_s[:, o:o + 128]

            def wt_v(ci):
                o = 1024 + ci * 512
                return wt_s[:, o:o + 512]

            # ---------------- state ----------------
            st = {"pe": 0.0, "act": 0.0, "dve": 0.0, "gp": 0.0}
            kq_ready = {}
            v_ready = {}
            exp_done = {}
            ex_tiles = {}
            sccopy_done = {}
            norm_done = {}
            den_jobs = []

            # ---------------- emitters ----------------
            qk_count = [0]

            def chain_qk(sp, ct, nb, dve_add=False):
                pg = pgen.tile([P, 512], f32, name="pg")
                wf = wt_q if sp == "q" else wt_k
                for ci in range(4):
                    MM(pg[:], wf(ci, ct), xt_s[:, nb, ci, :],
                       start=(ci == 0), stop=(ci == 3))
                st["pe"] += CHAIN4
                tok = nb * 512
                praw = prawp.tile([P, 512], f16, name="praw")
                qk_count[0] += 1
                if qk_count[0] <= 0:
                    # early phase: ACT is idle and DVE is the crunch
                    nc.scalar.activation(praw[:], pg[:], COPYF)
                    st["act"] = max(st["act"], st["pe"]) + VEVACA
                    st["dve"] = max(st["dve"], st["act"])
                else:
                    nc.vector.tensor_copy(praw[:], pg[:])
                t = tmps.tile([P, 2, 512], f16, name="t")
                nc.vector.tensor_mul(t[:, 0, :], praw[:], ce_s[:, tok:tok + 512])
                nc.vector.tensor_mul(t[:, 1, :], praw[:],
                                     ce_s[:, 2048 + tok:2560 + tok])
                ts = shfp.tile([P, 512], f16, name="ts")
                nc.vector.stream_shuffle(ts[:], t[:, 1, :], SWAPM)
                st["dve"] = max(st["dve"], st["pe"]) + PCAST + 2 * MULF + SHUF
                dest = qT[ct] if sp == "q" else kT[ct]
                if dve_add:
                    nc.vector.tensor_add(dest[:, tok:tok + 512], t[:, 0, :], ts[:])
                    st["dve"] += ADDF
                    kq_ready[(sp, ct, nb)] = st["dve"]
                else:
                    nc.gpsimd.tensor_add(dest[:, tok:tok + 512], t[:, 0, :], ts[:])
                    st["gp"] = max(st["gp"], st["dve"]) + GPADD
                    kq_ready[(sp, ct, nb)] = st["gp"]

            def chain_v(c):
                pg = pgen.tile([P, 512], f32, name="pg")
                nb, off = c // 4, (c % 4) * 128
                for ci in range(4):
                    MM(pg[:], xt_s[:, nb, ci, off:off + 128], wt_v(ci),
                       start=(ci == 0), stop=(ci == 3))
                st["pe"] += CHAIN4
                vv = v65[:, c, :, :]
                nc.scalar.activation(
                    vv[:, :, 0:DH],
                    pg[:].rearrange("p (h w) -> p h w", w=DH),
                    COPYF, scale=emask_s[:, c:c + 1])
                st["act"] = max(st["act"], st["pe"]) + VEVACA
                nc.vector.tensor_copy(
                    vv[:, :, DH:DH + 1],
                    emask_s[:, c:c + 1, None].to_broadcast((P, H, 1)))
                st["dve"] = max(st["dve"], st["pe"]) + 0.2
                v_ready[c] = max(st["act"], st["dve"])

            def s_war(p, c):
                hE = 2 * p
                return max(kq_ready.get(("q", p, 0), 0.0),
                           kq_ready.get(("q", p, 1), 0.0),
                           kq_ready.get(("k", p, c // 4), 0.0),
                           exp_done.get((hE, c - 1), 0.0))

            def emit_s_exp(p, c):
                kc = kT[p][:, c * 128:(c + 1) * 128]
                st["pe"] = max(st["pe"], s_war(p, c) - 0.45)
                psE = pss.tile([P, NQ], f32, name="ps")
                psO = pss.tile([P, NQ], f32, name="ps")
                for qb in (0, 1):
                    MM(psE[:, qb * 512:(qb + 1) * 512], kc[0:64, :],
                       qT[p][0:64, qb * 512:(qb + 1) * 512],
                       start=True, stop=True)
                st["pe"] += S_CH
                hE, hO = 2 * p, 2 * p + 1
                exE = exEp.tile([P, NQ], f16, name="exE")
                nc.scalar.activation(exE[:], psE[:], EXPF, scale=0.125)
                st["act"] = max(st["act"], st["pe"]) + EXPT
                exp_done[(hE, c)] = st["act"]
                ex_tiles[(hE, c)] = exE
                for qb in (0, 1):
                    MM(psO[:, qb * 512:(qb + 1) * 512], kc[64:128, :],
                       qT[p][64:128, qb * 512:(qb + 1) * 512],
                       start=True, stop=True)
                st["pe"] += S_CH
                exO = exOp.tile([P, NQ], f16, name="exO")
                nc.scalar.activation(exO[:], psO[:], EXPF, scale=0.125)
                st["act"] = max(st["act"], st["pe"] + EXPT, st["act"] + EXPT)
                exp_done[(hO, c)] = st["act"]
                ex_tiles[(hO, c)] = exO

            psav_t = [None]
            pg_o3 = [None, None]

            def emit_av(h, c):
                ex = ex_tiles.pop((h, c))
                if h == 7:
                    if c == 0:
                        pg_o3[0] = pgen.tile([P, 512], f32, name="pg")
                        pg_o3[1] = pgen.tile([P, 512], f32, name="pg")
                    for qb in (0, 1):
                        MM(pg_o3[qb][0:VW, :], v65[:, c, h, :],
                           ex[:, qb * 512:(qb + 1) * 512],
                           start=(c == 0), stop=(c == 15))
                else:
                    if c == 0:
                        psav_t[0] = psavp.tile([VW, NQ], f32, name="ps_o")
                    po = psav_t[0]
                    for qb in (0, 1):
                        MM(po[:, qb * 512:(qb + 1) * 512], v65[:, c, h, :],
                           ex[:, qb * 512:(qb + 1) * 512],
                           start=(c == 0), stop=(c == 15))
                st["pe"] += AV

            def start_epilogue(h):
                sc = scpool.tile([VW, NQ], f16, name="sc")
                if h == 7:
                    for qb in (0, 1):
                        nc.vector.tensor_copy(
                            sc[:, qb * 512:(qb + 1) * 512], pg_o3[qb][0:VW, :])
                    st["dve"] = max(st["dve"], st["pe"]) + SCC
                else:
                    po = psav_t[0]
                    nc.vector.tensor_copy(sc[:], po[:])
                    st["dve"] = max(st["dve"], st["pe"]) + SCC
                sccopy_done[h] = st["dve"]
                den_jobs.append({"h": h, "sc": sc, "ready": st["dve"]})

            def emit_den(job):
                h, sc = job["h"], job["sc"]
                p, off = h // 2, (h % 2) * 64
                st["pe"] = max(st["pe"], job["ready"])
                ones = bias_s[64:65, 512:576]
                rr = rrp.tile([DH, NQ], f32, name="rr")
                if h < 4:
                    pd0 = pgen.tile([P, 512], f32, name="pg")
                    pd1 = pgen.tile([P, 512], f32, name="pg")
                    MM(pd0[0:64, :], ones, sc[64:65, 0:512], start=True, stop=True)
                    MM(pd1[0:64, :], ones, sc[64:65, 512:1024], start=True, stop=True)
                    st["pe"] += DEN2
                    nc.vector.reciprocal_approx_fast(rr[:, 0:512], pd0[0:64, :])
                    nc.vector.reciprocal_approx_fast(rr[:, 512:1024], pd1[0:64, :])
                else:
                    # qb-pipelined so late consumers unblock on half ranges
                    pd = pss.tile([P, NQ], f32, name="ps")
                    for qb in (0, 1):
                        sl = slice(qb * 512, (qb + 1) * 512)
                        MM(pd[0:64, sl], ones, sc[64:65, sl], start=True, stop=True)
                        st["pe"] += DEN2 / 2
                        nc.vector.reciprocal_approx_fast(rr[:, sl], pd[0:64, sl])
                        nc.vector.tensor_mul(scp[p][off:off + 64, sl],
                                             sc[0:64, sl], rr[:, sl])
                    st["dve"] = max(st["dve"], st["pe"]) + RECIP + NMUL
                    norm_done[h] = st["dve"]
                    return
                nc.vector.tensor_mul(scp[p][off:off + 64, :], sc[0:64, :], rr[:])
                st["dve"] = max(st["dve"], st["pe"]) + RECIP + NMUL
                norm_done[h] = st["dve"]

            def emit_opA(nbk):
                pg = pgen.tile([P, 512], f32, name="pg")
                MM(pg[:], scp[0][:, nbk * 128:(nbk + 1) * 128],
                   wpt_s[:, 0:512], start=True, stop=False)
                MM(pg[:], scp[1][:, nbk * 128:(nbk + 1) * 128],
                   wpt_s[:, 512:1024], start=False, stop=True)
                st["pe"] += OP2
                nc.vector.tensor_add(ysum[:, nbk, :], pg[:], bias_s[:, 0:512])
                st["dve"] = max(st["dve"], st["pe"]) + NMUL

            def emit_tail(nbk, sl):
                MM(sl, scp[2][:, nbk * 128:(nbk + 1) * 128],
                   wpt_s[:, 1024:1536], start=True, stop=False)
                MM(sl, scp[3][:, nbk * 128:(nbk + 1) * 128],
                   wpt_s[:, 1536:2048], start=False, stop=True)
                st["pe"] += OP2
                nc.vector.tensor_add(y_all[:, nbk, :], ysum[:, nbk, :], sl)
                st["dve"] = max(st["dve"], st["pe"]) + NMUL

            # ---------------- schedule ----------------
            # HAM warm-up: dummy matmuls on garbage SBUF during the DMA
            # lead-in so the PE clock gate opens (1.2 -> 2.4 GHz) before the
            # first real chain; plus a dummy ACTIVATE to pull the ~2.7us
            # ACT table load off the critical path.
            warm = tmps.tile([P, 2, 512], f16, name="t")
            wps = pss.tile([P, NQ], f32, name="ps")
            for i in range(10):
                MM(wps[:, 0:512], warm[:, 0, 0:128], warm[:, 1, :],
                   start=True, stop=True)
            nc.scalar.activation(warm[:, 0, 0:16], wps[:, 0:16], EXPF,
                                 scale=0.125)
            chain_qk("q", 0, 0, dve_add=True)
            chain_qk("k", 0, 0, dve_add=True)
            chain_qk("q", 0, 1, dve_add=True)

            filler = [("k", 0, 1), ("v", 0), ("k", 0, 2), ("v", 1),
                      ("k", 0, 3), ("v", 2), ("q", 1, 0), ("v", 3),
                      ("q", 1, 1), ("v", 4), ("k", 1, 0), ("v", 5),
                      ("k", 1, 1), ("v", 6), ("k", 1, 2), ("v", 7),
                      ("k", 1, 3), ("v", 8), ("v", 9), ("q", 2, 0),
                      ("v", 10), ("q", 2, 1), ("v", 11), ("k", 2, 0),
                      ("v", 12), ("k", 2, 1), ("v", 13), ("k", 2, 2),
                      ("v", 14), ("k", 2, 3), ("v", 15), ("q", 3, 0),
                      ("q", 3, 1), ("k", 3, 0), ("k", 3, 1), ("k", 3, 2),
                      ("k", 3, 3)]
            fpos = {it: i for i, it in enumerate(filler)}
            fidx = [0]

            def consume_one():
                it = filler[fidx[0]]
                fidx[0] += 1
                if it[0] == "v":
                    chain_v(it[1])
                else:
                    chain_qk(*it)

            def consume_until(i):
                while fidx[0] <= min(i, len(filler) - 1):
                    consume_one()

            def req_s(p, c):
                mx = -1
                for key in (("q", p, 0), ("q", p, 1), ("k", p, c // 4)):
                    if key in fpos:
                        mx = max(mx, fpos[key])
                return mx

            avq = []
            for p in range(3):
                avq += [(2 * p, c) for c in range(NCH)]
                avq += [(2 * p + 1, c) for c in range(NCH)]
            for c in range(NCH):
                avq += [(6, c), (7, c)]
            avi = [0]
            # psav users in order; each gated on predecessor's sccopy
            psav_prev = {0: None, 1: 0, 2: 1, 3: 2, 4: 3, 5: 4, 6: 5}
            opA_q = []
            opA_state = [0]  # 0=locked, 1=unlocked

            def av_ready(slack):
                if avi[0] >= len(avq):
                    return False
                h, c = avq[avi[0]]
                if exp_done.get((h, c), 1e9) > st["pe"] + slack:
                    return False
                if v_ready.get(c, 1e9) > st["pe"] + slack:
                    return False
                if c == 0:
                    if h == 7:
                        # pgen claim: all pgen users must be emitted first
                        if fidx[0] < len(filler) or opA_q or opA_state[0] == 0:
                            return False
                        if any(j["h"] < 4 for j in den_jobs):
                            return False
                    else:
                        prev = psav_prev[h]
                        if prev is not None and \
                                sccopy_done.get(prev, 1e9) > st["pe"] + slack:
                            return False
                return True

            def try_unlock_opA():
                if opA_state[0] == 0 and norm_done.get(3, 1e9) <= st["dve"]:
                    opA_state[0] = 1
                    opA_q.extend(range(8))

            def emit_one_av():
                h, c = avq[avi[0]]
                avi[0] += 1
                emit_av(h, c)
                if c == 15:
                    start_epilogue(h)

            for s in range(64):
                p, c = divmod(s, NCH)
                # paced filler: walk toward the requirement 4 periods out,
                # at most 2 chains per period (avoids bursts that stall EXP)
                if s >= 2:
                    tgt = req_s(*divmod(min(s + 6, 63), NCH))
                    n = 0
                    while fidx[0] <= tgt and n < 2:
                        consume_one()
                        n += 1
                consume_until(req_s(p, c))
                emit_s_exp(p, c)
                while den_jobs and den_jobs[0]["ready"] <= st["pe"] + 0.6:
                    emit_den(den_jobs.pop(0))
                try_unlock_opA()
                # attnv: keep pace with EXP production (2 head-chunks per
                # period) plus one catch-up when the backlog grows
                backlog = 2 * (s + 1) - avi[0]
                nav = 2 + (1 if backlog > 8 else 0) + (1 if backlog > 16 else 0)
                navper = 0
                while navper < nav and av_ready(0.3):
                    emit_one_av()
                    navper += 1
                while st["pe"] < st["act"] - 0.1:
                    if av_ready(0.3):
                        emit_one_av()
                    elif den_jobs and den_jobs[0]["ready"] <= st["pe"]:
                        emit_den(den_jobs.pop(0))
                    elif opA_q:
                        emit_opA(opA_q.pop(0))
                        try_unlock_opA()
                    elif fidx[0] < len(filler):
                        consume_one()
                    else:
                        break

            # ---------------- drain ----------------
            guard = 0
            while avi[0] < len(avq):
                guard += 1
                assert guard < 10000, "drain stall"
                try_unlock_opA()
                if av_ready(0.3):
                    emit_one_av()
                elif den_jobs and den_jobs[0]["ready"] <= st["pe"] + 0.3:
                    emit_den(den_jobs.pop(0))
                elif fidx[0] < len(filler):
                    consume_one()
                elif opA_q and opA_state[0] == 1:
                    emit_opA(opA_q.pop(0))
                else:
                    st["pe"] += 0.25  # idle nudge toward blocking dep
            while fidx[0] < len(filler):
                consume_one()
            # drain dummies: keep HAM warm across the DVE-gated den chains
            psd = [None]

            def dummy_mm():
                if psd[0] is None:
                    psd[0] = psavp.tile([VW, NQ], f32, name="ps_o")
                MM(psd[0][0:VW, 0:512], gb[:, 0, 0:VW], gb[:, 1, :],
                   start=True, stop=True)
                st["pe"] += 0.25
            while den_jobs:
                while den_jobs[0]["ready"] > st["pe"] + 0.3:
                    dummy_mm()
                emit_den(den_jobs.pop(0))
            try_unlock_opA()
            while opA_q:
                emit_opA(opA_q.pop(0))

            for g in range(4):
                pt = pss.tile([P, NQ], f32, name="ps")
                emit_tail(2 * g, pt[:, 0:512])
                emit_tail(2 * g + 1, pt[:, 512:1024])
                DMA(y_d.ap()[:, g * 1024:(g + 1) * 1024],
                    y_all[:, 2 * g:2 * g + 2, :])

    nc.compile()
    return nc


def _get_module():
    if "nc" not in _CACHE:
        _CACHE["nc"] = _build_module()
    return _CACHE["nc"]


def kernel(x, mask, times, Wqkv, Wproj, bproj, num_cls_token=0, _trace=False):
    from concourse.bass_utils import run_bass_kernel_spmd

    assert int(num_cls_token) == 0, "kernel specialized for num_cls_token=0"
    in_maps = _host_prep(x, mask, times, Wqkv, Wproj, bproj)
    nc = _get_module()
    res = run_bass_kernel_spmd(nc, in_maps, list(range(8)), trace=_trace)
    _CACHE["last_result"] = res

    out = np.empty((B, N, C), np.float32)
    for core in range(8):
        b, qhalf = core // 2, core % 2
        y = np.asarray(res.results[core]["y"], np.float32)   # [128, 4096]
        blk = y.reshape(P, 8, 512).transpose(1, 0, 2).reshape(NQ, C)
        out[b, qhalf * NQ:(qhalf + 1) * NQ, :] = blk
    return out
